# revision 16
# baseline (speedup 1.0000x reference)
"""MoE gate (top-2 of 64 experts) Trainium2 Bass kernel.

Problem: hidden_states [4, 4096, 2048] f32, gate weight [64, 2048] f32.
  logits = x @ W.T            [16384, 64]
  scores = softmax(logits)
  topk_w, topk_i = top_k(scores, 2); topk_w normalized by their sum
  aux_loss from per-batch expert counts (ce) and mean scores.

Sharding: data-parallel over batch*seq. 16384 rows -> 2048 rows/core on 8
cores; the [64, 2048] gate weight is replicated (passed pre-transposed).
x is passed per-core pre-transposed ([D, rows] layout) so the contraction
dim D lands on SBUF partitions with fully-contiguous DMA lines.

Per core device program (Tile framework):
  4 stages x 512 rows. Per stage: 16 k-chunk DMAs [128, 512] feed
  fp32 matmuls accumulating logits into one PSUM tile [128, 4*64]
  (4 row-tiles of 128 rows side by side). Stats: DVE max8/max_index give
  top-2 values+indices per row; ACT exp(logits - max) with accum_out gives
  e and its row-sum; softmax column-sums for the aux loss accumulate on the
  PE via a [128,1] x [128,64] matmul with 1/rowsum as the stationary.
  ce and the final scalar aux loss are reduced on host from returned
  per-core indices and score sums (tiny [8,64] + [16384,2] tensors).
"""

import numpy as np
from contextlib import ExitStack

import concourse.bass as bass
import concourse.tile as tile
from concourse import bacc, mybir
from concourse.bass_utils import run_bass_kernel_spmd

# problem constants (hardcoded per harness contract)
B, S, D, E = 4, 4096, 2048, 64
TOP_K = 2
ALPHA = 0.01
N_CORES = 8
R = (B * S) // N_CORES  # 2048 rows per core
KC = D // 128           # 16 contraction chunks
STAGES = 4
SR = R // STAGES        # 512 rows per stage
JT = SR // 128          # 4 row-tiles per stage

F32 = mybir.dt.float32
U32 = mybir.dt.uint32


def build_moe_gate_kernel():
    nc = bacc.Bacc("TRN2", target_bir_lowering=False, debug=False)

    BF16 = mybir.dt.bfloat16
    xhiT = nc.dram_tensor("xhiT", [D, R], BF16, kind="ExternalInput").ap()
    xloT = nc.dram_tensor("xloT", [D, R], BF16, kind="ExternalInput").ap()
    # stacked gate weight [Whi | Wlo] transposed: [D, 2*E] bf16
    wS = nc.dram_tensor("wS", [D, 2 * E], BF16, kind="ExternalInput").ap()
    idx_out = nc.dram_tensor("idx_out", [R, TOP_K], U32, kind="ExternalOutput").ap()
    w_out = nc.dram_tensor("w_out", [R, TOP_K], F32, kind="ExternalOutput").ap()
    ssum_out = nc.dram_tensor("ssum_out", [1, E], F32, kind="ExternalOutput").ap()

    F32R = mybir.dt.float32r

    with tile.TileContext(nc) as tc, ExitStack() as ctx:
        wpool = ctx.enter_context(tc.tile_pool(name="w", bufs=1))
        xpool = ctx.enter_context(tc.tile_pool(name="x", bufs=8))
        # PSUM banks: 2 lgT (raw [64,512] logits) + 2 lg2 (transposed) + 1 ssum
        lgtpool = ctx.enter_context(tc.tile_pool(name="lgt", bufs=2, space="PSUM"))
        lg2pool = ctx.enter_context(tc.tile_pool(name="lg2", bufs=2, space="PSUM"))
        sspool = ctx.enter_context(tc.tile_pool(name="ss", bufs=1, space="PSUM"))
        spool = ctx.enter_context(tc.tile_pool(name="st", bufs=2))
        epool = ctx.enter_context(tc.tile_pool(name="e", bufs=2 * JT))

        # stacked gate weight, chunk k at [:, k*2E:(k+1)*2E] (bf16, FWL-able)
        wt_sb = wpool.tile([128, KC * 2 * E], BF16)
        nc.sync.dma_start(wt_sb[:], wS.rearrange("(k p) e -> p k e", p=128))

        # identity for the PE transpose-back of full [128, 128] blocks
        ident_dram = nc.inline_tensor(np.eye(128, dtype=np.float32), name="ident128")
        ident = wpool.tile([128, 128], F32)
        nc.sync.dma_start(ident[:], ident_dram.ap())

        # whole-kernel softmax column-sum accumulator; allocated full-height
        # so the bank isn't shared with (and zeroed under) the logits tiles
        ssum_t = sspool.tile([128, E], F32)
        ssum = ssum_t[0:1, :]

        def emit_mm(s):
            # logits.T for this stage via 2x2 bf16 split: stationary is the
            # stacked [Whi | Wlo] chunk (128 cols, one FWL load per k); the
            # hi and lo row-streams go through it back-to-back at N=512.
            # PSUM [128, 512]: rows 0:64 = Whi^T x*, rows 64:128 = Wlo^T x*.
            lgT = lgtpool.tile([128, SR], F32, tag="lgT", name=f"lgT{s}")
            for k in range(KC):
                xh = xpool.tile([128, SR], BF16, tag="xk", name=f"xh{s}_{k}")
                nc.sync.dma_start(
                    xh[:], xhiT[k * 128 : (k + 1) * 128, s * SR : (s + 1) * SR]
                )
                xl = xpool.tile([128, SR], BF16, tag="xk", name=f"xl{s}_{k}")
                nc.sync.dma_start(
                    xl[:], xloT[k * 128 : (k + 1) * 128, s * SR : (s + 1) * SR]
                )
                stk = wt_sb[:, k * 2 * E : (k + 1) * 2 * E]
                nc.tensor.matmul(
                    lgT[:], lhsT=stk, rhs=xh[:], start=(k == 0), stop=False
                )
                nc.tensor.matmul(
                    lgT[:], lhsT=stk, rhs=xl[:], start=False, stop=(k == KC - 1)
                )
            return lgT

        def emit_stats(s, lgT):
            # PSUM -> SBUF; the Whi (rows 0:64) and Wlo (rows 64:128) halves
            # are summed for free by PSUM accumulation across the two
            # transposes-back to [128 rows, 64 experts]
            lgsb = spool.tile([128, SR], F32, tag="lgsb", name=f"lgsb{s}")
            nc.vector.tensor_copy(lgsb[:], lgT[:])
            # transpose whole [128, 128] blocks: row-tile j's transposed block
            # has hi-logits in cols 0:64 and lo-logits in cols 64:128
            lg2 = lg2pool.tile([128, JT * 128], F32, tag="lg2", name=f"lg2_{s}")
            for j in range(JT):
                nc.tensor.matmul(
                    lg2[:, j * 128 : (j + 1) * 128],
                    lhsT=lgsb[:, j * 128 : (j + 1) * 128],
                    rhs=ident[:],
                    is_transpose=True,
                    start=True,
                    stop=True,
                )
            lg2sb = spool.tile([128, JT * 128], F32, tag="lg2sb", name=f"lg2sb{s}")
            nc.vector.tensor_copy(lg2sb[:], lg2[:])
            # hi + lo halves summed in one strided add -> [128, JT*64] logits
            h3 = lg2sb.rearrange("p (j two e) -> p j two e", two=2, e=E)
            lgs = spool.tile([128, JT * E], F32, tag="lgs", name=f"lgs{s}")
            l3 = lgs.rearrange("p (j e) -> p j e", e=E)
            nc.vector.tensor_add(l3[:, :, :], h3[:, :, 0, :], h3[:, :, 1, :])

            vals8 = spool.tile([128, JT * 8], F32, tag="v8", name=f"v8_{s}")
            idx8 = spool.tile([128, JT * 8], U32, tag="i8", name=f"i8_{s}")
            for j in range(JT):
                nc.vector.max(vals8[:, j * 8 : (j + 1) * 8], lgs[:, j * E : (j + 1) * E])
                nc.vector.max_index(
                    idx8[:, j * 8 : (j + 1) * 8],
                    vals8[:, j * 8 : (j + 1) * 8],
                    lgs[:, j * E : (j + 1) * E],
                )

            v3 = vals8.rearrange("p (j c) -> p j c", c=8)
            l1 = v3[:, :, 0:1]  # [128, JT, 1] top-1 logit
            l2 = v3[:, :, 1:2]  # top-2 logit

            negm1 = spool.tile([128, JT], F32, tag="nm", name=f"nm_{s}")
            nc.vector.tensor_scalar_mul(negm1[:], l1, -1.0)
            d21 = spool.tile([128, JT], F32, tag="d21", name=f"d21_{s}")
            nc.vector.tensor_sub(d21[:], l2, l1)

            denom = spool.tile([128, JT], F32, tag="dn", name=f"dn_{s}")
            ev = []
            for j in range(JT):
                e_j = epool.tile([128, E], F32, tag="ev", name=f"ev_{s}_{j}")
                nc.scalar.activation(
                    e_j[:],
                    lgs[:, j * E : (j + 1) * E],
                    mybir.ActivationFunctionType.Exp,
                    bias=negm1[:, j : j + 1],
                    scale=1.0,
                    accum_out=denom[:, j : j + 1],
                )
                ev.append(e_j)

            recip = spool.tile([128, JT], F32, tag="rc", name=f"rc_{s}")
            nc.vector.reciprocal(recip[:], denom[:])

            # e2 = exp(l2 - l1); w1 = 1/(1+e2); w2 = e2 * w1
            e2 = spool.tile([128, JT], F32, tag="e2", name=f"e2_{s}")
            nc.scalar.activation(e2[:], d21[:], mybir.ActivationFunctionType.Exp)
            den2 = spool.tile([128, JT], F32, tag="dn2", name=f"dn2_{s}")
            nc.vector.tensor_scalar_add(den2[:], e2[:], 1.0)
            wq = spool.tile([128, JT * 2], F32, tag="wq", name=f"wq_{s}")
            wq3 = wq.rearrange("p (j c) -> p j c", c=2)
            nc.vector.reciprocal(wq3[:, :, 0:1], den2[:])
            nc.vector.tensor_mul(wq3[:, :, 1:2], e2[:], wq3[:, :, 0:1])

            # scores column-sum: ssum += recip_j^T @ e_j  (over all stages)
            for j in range(JT):
                nc.tensor.matmul(
                    ssum[:],
                    lhsT=recip[:, j : j + 1],
                    rhs=ev[j][:],
                    start=(s == 0 and j == 0),
                    stop=(s == STAGES - 1 and j == JT - 1),
                )

            # outputs: rows s*SR + j*128 + p, cols c
            i3 = idx8.rearrange("p (j c) -> p j c", c=8)
            dst_i = idx_out[s * SR : (s + 1) * SR, :].rearrange(
                "(j p) c -> p j c", p=128
            )
            nc.sync.dma_start(dst_i, i3[:, :, 0:TOP_K])
            dst_w = w_out[s * SR : (s + 1) * SR, :].rearrange(
                "(j p) c -> p j c", p=128
            )
            nc.sync.dma_start(dst_w, wq3[:, :, :])

        # software pipeline: stage s matmuls run while stage s-1 stats drain,
        # keeping the PE stream dense across stage boundaries
        lgT_prev = emit_mm(0)
        for s in range(1, STAGES):
            lgT_cur = emit_mm(s)
            emit_stats(s - 1, lgT_prev)
            lgT_prev = lgT_cur
        emit_stats(STAGES - 1, lgT_prev)

        # final: score sums to DRAM (via SBUF; DMA from PSUM is restricted)
        ssum_sb = spool.tile([1, E], F32)
        nc.vector.tensor_copy(ssum_sb[:], ssum[:])
        nc.sync.dma_start(ssum_out, ssum_sb[:])

    nc.compile()
    return nc


_NC_CACHE = None

# test-harness knobs (harness never touches these; kernel() defaults are fine)
TRACE = False
TMPDIR = None
LAST_RESULT = None


def _get_nc():
    global _NC_CACHE
    if _NC_CACHE is None:
        _NC_CACHE = build_moe_gate_kernel()
    return _NC_CACHE


def kernel(hidden_states: np.ndarray, weight: np.ndarray):
    global LAST_RESULT
    import ml_dtypes

    bf16 = ml_dtypes.bfloat16
    nc = _get_nc()
    x = np.asarray(hidden_states, dtype=np.float32).reshape(B * S, D)
    w = np.asarray(weight, dtype=np.float32)

    # 2x2 bf16 split: v = hi + lo with hi = bf16(v), lo = bf16(v - hi);
    # [Whi | Wlo] stacked so one 128-col stationary serves both terms
    w_hi = w.astype(bf16)
    w_lo = (w - w_hi.astype(np.float32)).astype(bf16)
    wS_np = np.ascontiguousarray(
        np.concatenate([w_hi.T, w_lo.T], axis=1)
    )  # [D, 2E] bf16

    x_hi = x.astype(bf16)
    x_lo = (x - x_hi.astype(np.float32)).astype(bf16)

    in_maps = []
    for c in range(N_CORES):
        rows = slice(c * R, (c + 1) * R)
        in_maps.append(
            {
                "xhiT": np.ascontiguousarray(x_hi[rows].T),
                "xloT": np.ascontiguousarray(x_lo[rows].T),
                "wS": wS_np,
            }
        )

    res = run_bass_kernel_spmd(
        nc, in_maps, list(range(N_CORES)), trace=TRACE, tmpdir=TMPDIR
    )
    LAST_RESULT = res
    results = res.results

    idx = np.concatenate([results[c]["idx_out"] for c in range(N_CORES)], axis=0)
    idx = idx.astype(np.int32)  # values 0..63; uint32 -> int32 exact
    tw = np.concatenate([results[c]["w_out"] for c in range(N_CORES)], axis=0)
    ssum = np.stack([results[c]["ssum_out"][0] for c in range(N_CORES)])  # [8, E]

    # host-side tiny reductions for the aux loss
    cores_per_batch = N_CORES // B  # 2
    mean_scores = np.zeros((B, E), np.float32)
    for b in range(B):
        mean_scores[b] = (
            ssum[b * cores_per_batch : (b + 1) * cores_per_batch].sum(axis=0) / S
        )
    idx_b = idx.reshape(B, S * TOP_K)
    ce = np.zeros((B, E), np.float32)
    for b in range(B):
        ce[b] = np.bincount(idx_b[b], minlength=E).astype(np.float32)
    ce /= S * TOP_K / E
    aux_loss = np.float32((ce * mean_scores).sum(axis=1).mean() * ALPHA)

    return idx, tw, aux_loss


# revision 17
# speedup vs baseline: 1.2873x; 1.2873x over previous
"""MoE gate (top-2 of 64 experts) Trainium2 Bass kernel.

Problem: hidden_states [4, 4096, 2048] f32, gate weight [64, 2048] f32.
  logits = x @ W.T            [16384, 64]
  scores = softmax(logits)
  topk_w, topk_i = top_k(scores, 2); topk_w normalized by their sum
  aux_loss from per-batch expert counts (ce) and mean scores.

Sharding: data-parallel over batch*seq. 16384 rows -> 2048 rows/core on 8
cores; the [64, 2048] gate weight is replicated (passed pre-transposed).
x is passed per-core pre-transposed ([D, rows] layout) so the contraction
dim D lands on SBUF partitions with fully-contiguous DMA lines.

Per core device program (Tile framework):
  4 stages x 512 rows. Per stage: 16 k-chunk DMAs [128, 512] feed
  fp32 matmuls accumulating logits into one PSUM tile [128, 4*64]
  (4 row-tiles of 128 rows side by side). Stats: DVE max8/max_index give
  top-2 values+indices per row; ACT exp(logits - max) with accum_out gives
  e and its row-sum; softmax column-sums for the aux loss accumulate on the
  PE via a [128,1] x [128,64] matmul with 1/rowsum as the stationary.
  ce and the final scalar aux loss are reduced on host from returned
  per-core indices and score sums (tiny [8,64] + [16384,2] tensors).
"""

import numpy as np
from contextlib import ExitStack

import concourse.bass as bass
import concourse.tile as tile
from concourse import bacc, mybir
from concourse.bass_utils import run_bass_kernel_spmd

# problem constants (hardcoded per harness contract)
B, S, D, E = 4, 4096, 2048, 64
TOP_K = 2
ALPHA = 0.01
N_CORES = 8
R = (B * S) // N_CORES  # 2048 rows per core
KC = D // 128           # 16 contraction chunks
STAGES = 4
SR = R // STAGES        # 512 rows per stage
JT = SR // 128          # 4 row-tiles per stage

F32 = mybir.dt.float32
U32 = mybir.dt.uint32


def build_moe_gate_kernel():
    nc = bacc.Bacc("TRN2", target_bir_lowering=False, debug=False)

    BF16 = mybir.dt.bfloat16
    # hi/lo bf16 split packed per d-row: [D, 2, R] -> one 1MB DMA per k-chunk
    # with 8KB contiguous partition lines
    xsT = nc.dram_tensor("xsT", [D, 2, R], BF16, kind="ExternalInput").ap()
    # stacked gate weight [Whi | Wlo] transposed: [D, 2*E] bf16
    wS = nc.dram_tensor("wS", [D, 2 * E], BF16, kind="ExternalInput").ap()
    idx_out = nc.dram_tensor("idx_out", [R, TOP_K], U32, kind="ExternalOutput").ap()
    w_out = nc.dram_tensor("w_out", [R, TOP_K], F32, kind="ExternalOutput").ap()
    ssum_out = nc.dram_tensor("ssum_out", [1, E], F32, kind="ExternalOutput").ap()

    F32R = mybir.dt.float32r

    with tile.TileContext(nc) as tc, ExitStack() as ctx:
        wpool = ctx.enter_context(tc.tile_pool(name="w", bufs=1))
        xpool = ctx.enter_context(tc.tile_pool(name="x", bufs=KC))
        # PSUM banks: 2 lgT (raw [64,512] logits) + 2 lg2 (transposed) + 1 ssum
        lgtpool = ctx.enter_context(tc.tile_pool(name="lgt", bufs=2, space="PSUM"))
        lg2pool = ctx.enter_context(tc.tile_pool(name="lg2", bufs=2, space="PSUM"))
        sspool = ctx.enter_context(tc.tile_pool(name="ss", bufs=1, space="PSUM"))
        spool = ctx.enter_context(tc.tile_pool(name="st", bufs=2))
        epool = ctx.enter_context(tc.tile_pool(name="e", bufs=2 * JT))

        # stacked gate weight, chunk k at [:, k*2E:(k+1)*2E] (bf16, FWL-able)
        wt_sb = wpool.tile([128, KC * 2 * E], BF16)
        nc.sync.dma_start(wt_sb[:], wS.rearrange("(k p) e -> p k e", p=128))

        # identity for the PE transpose-back of full [128, 128] blocks
        ident_dram = nc.inline_tensor(np.eye(128, dtype=np.float32), name="ident128")
        ident = wpool.tile([128, 128], F32)
        nc.sync.dma_start(ident[:], ident_dram.ap())

        # whole-kernel softmax column-sum accumulator; allocated full-height
        # so the bank isn't shared with (and zeroed under) the logits tiles
        ssum_t = sspool.tile([128, E], F32)
        ssum = ssum_t[0:1, :]

        # all 16 k-chunk DMAs issued up front: 1MB each, 8KB/partition lines;
        # the full split x (16MB) resides in SBUF
        xks = []
        for k in range(KC):
            xk = xpool.tile([128, 2 * R], BF16, tag="xk", name=f"xk_{k}")
            nc.sync.dma_start(xk[:], xsT[k * 128 : (k + 1) * 128, :, :])
            xks.append(xk)

        def emit_mm(s):
            # logits.T for this stage via 2x2 bf16 split: stationary is the
            # stacked [Whi | Wlo] chunk (128 cols, one FWL load per k); the
            # hi and lo row-streams go through it back-to-back at N=512.
            # PSUM [128, 512]: rows 0:64 = Whi^T x*, rows 64:128 = Wlo^T x*.
            lgT = lgtpool.tile([128, SR], F32, tag="lgT", name=f"lgT{s}")
            for k in range(KC):
                stk = wt_sb[:, k * 2 * E : (k + 1) * 2 * E]
                xh = xks[k][:, s * SR : (s + 1) * SR]
                xl = xks[k][:, R + s * SR : R + (s + 1) * SR]
                nc.tensor.matmul(
                    lgT[:], lhsT=stk, rhs=xh, start=(k == 0), stop=False
                )
                nc.tensor.matmul(
                    lgT[:], lhsT=stk, rhs=xl, start=False, stop=(k == KC - 1)
                )
            return lgT

        def emit_stats(s, lgT):
            # PSUM -> SBUF; the Whi (rows 0:64) and Wlo (rows 64:128) halves
            # are summed for free by PSUM accumulation across the two
            # transposes-back to [128 rows, 64 experts]
            lgsb = spool.tile([128, SR], F32, tag="lgsb", name=f"lgsb{s}")
            nc.vector.tensor_copy(lgsb[:], lgT[:])
            # transpose whole [128, 128] blocks: row-tile j's transposed block
            # has hi-logits in cols 0:64 and lo-logits in cols 64:128
            lg2 = lg2pool.tile([128, JT * 128], F32, tag="lg2", name=f"lg2_{s}")
            for j in range(JT):
                nc.tensor.matmul(
                    lg2[:, j * 128 : (j + 1) * 128],
                    lhsT=lgsb[:, j * 128 : (j + 1) * 128],
                    rhs=ident[:],
                    is_transpose=True,
                    start=True,
                    stop=True,
                )
            lg2sb = spool.tile([128, JT * 128], F32, tag="lg2sb", name=f"lg2sb{s}")
            nc.vector.tensor_copy(lg2sb[:], lg2[:])
            # hi + lo halves summed in one strided add -> [128, JT*64] logits
            h3 = lg2sb.rearrange("p (j two e) -> p j two e", two=2, e=E)
            lgs = spool.tile([128, JT * E], F32, tag="lgs", name=f"lgs{s}")
            l3 = lgs.rearrange("p (j e) -> p j e", e=E)
            nc.vector.tensor_add(l3[:, :, :], h3[:, :, 0, :], h3[:, :, 1, :])

            vals8 = spool.tile([128, JT * 8], F32, tag="v8", name=f"v8_{s}")
            idx8 = spool.tile([128, JT * 8], U32, tag="i8", name=f"i8_{s}")
            for j in range(JT):
                nc.vector.max(vals8[:, j * 8 : (j + 1) * 8], lgs[:, j * E : (j + 1) * E])
                nc.vector.max_index(
                    idx8[:, j * 8 : (j + 1) * 8],
                    vals8[:, j * 8 : (j + 1) * 8],
                    lgs[:, j * E : (j + 1) * E],
                )

            v3 = vals8.rearrange("p (j c) -> p j c", c=8)
            l1 = v3[:, :, 0:1]  # [128, JT, 1] top-1 logit
            l2 = v3[:, :, 1:2]  # top-2 logit

            negm1 = spool.tile([128, JT], F32, tag="nm", name=f"nm_{s}")
            nc.vector.tensor_scalar_mul(negm1[:], l1, -1.0)
            d21 = spool.tile([128, JT], F32, tag="d21", name=f"d21_{s}")
            nc.vector.tensor_sub(d21[:], l2, l1)

            denom = spool.tile([128, JT], F32, tag="dn", name=f"dn_{s}")
            ev = []
            for j in range(JT):
                e_j = epool.tile([128, E], F32, tag="ev", name=f"ev_{s}_{j}")
                nc.scalar.activation(
                    e_j[:],
                    lgs[:, j * E : (j + 1) * E],
                    mybir.ActivationFunctionType.Exp,
                    bias=negm1[:, j : j + 1],
                    scale=1.0,
                    accum_out=denom[:, j : j + 1],
                )
                ev.append(e_j)

            recip = spool.tile([128, JT], F32, tag="rc", name=f"rc_{s}")
            nc.vector.reciprocal(recip[:], denom[:])

            # e2 = exp(l2 - l1); w1 = 1/(1+e2); w2 = e2 * w1
            e2 = spool.tile([128, JT], F32, tag="e2", name=f"e2_{s}")
            nc.scalar.activation(e2[:], d21[:], mybir.ActivationFunctionType.Exp)
            den2 = spool.tile([128, JT], F32, tag="dn2", name=f"dn2_{s}")
            nc.vector.tensor_scalar_add(den2[:], e2[:], 1.0)
            wq = spool.tile([128, JT * 2], F32, tag="wq", name=f"wq_{s}")
            wq3 = wq.rearrange("p (j c) -> p j c", c=2)
            nc.vector.reciprocal(wq3[:, :, 0:1], den2[:])
            nc.vector.tensor_mul(wq3[:, :, 1:2], e2[:], wq3[:, :, 0:1])

            # scores column-sum: ssum += recip_j^T @ e_j  (over all stages)
            for j in range(JT):
                nc.tensor.matmul(
                    ssum[:],
                    lhsT=recip[:, j : j + 1],
                    rhs=ev[j][:],
                    start=(s == 0 and j == 0),
                    stop=(s == STAGES - 1 and j == JT - 1),
                )

            # outputs: rows s*SR + j*128 + p, cols c
            i3 = idx8.rearrange("p (j c) -> p j c", c=8)
            dst_i = idx_out[s * SR : (s + 1) * SR, :].rearrange(
                "(j p) c -> p j c", p=128
            )
            nc.sync.dma_start(dst_i, i3[:, :, 0:TOP_K])
            dst_w = w_out[s * SR : (s + 1) * SR, :].rearrange(
                "(j p) c -> p j c", p=128
            )
            nc.sync.dma_start(dst_w, wq3[:, :, :])

        # software pipeline: stage s matmuls run while stage s-1 stats drain,
        # keeping the PE stream dense across stage boundaries
        lgT_prev = emit_mm(0)
        for s in range(1, STAGES):
            lgT_cur = emit_mm(s)
            emit_stats(s - 1, lgT_prev)
            lgT_prev = lgT_cur
        emit_stats(STAGES - 1, lgT_prev)

        # final: score sums to DRAM (via SBUF; DMA from PSUM is restricted)
        ssum_sb = spool.tile([1, E], F32)
        nc.vector.tensor_copy(ssum_sb[:], ssum[:])
        nc.sync.dma_start(ssum_out, ssum_sb[:])

    nc.compile()
    return nc


_NC_CACHE = None

# test-harness knobs (harness never touches these; kernel() defaults are fine)
TRACE = False
TMPDIR = None
LAST_RESULT = None


def _get_nc():
    global _NC_CACHE
    if _NC_CACHE is None:
        _NC_CACHE = build_moe_gate_kernel()
    return _NC_CACHE


def kernel(hidden_states: np.ndarray, weight: np.ndarray):
    global LAST_RESULT
    import ml_dtypes

    bf16 = ml_dtypes.bfloat16
    nc = _get_nc()
    x = np.asarray(hidden_states, dtype=np.float32).reshape(B * S, D)
    w = np.asarray(weight, dtype=np.float32)

    # 2x2 bf16 split: v = hi + lo with hi = bf16(v), lo = bf16(v - hi);
    # [Whi | Wlo] stacked so one 128-col stationary serves both terms
    w_hi = w.astype(bf16)
    w_lo = (w - w_hi.astype(np.float32)).astype(bf16)
    wS_np = np.ascontiguousarray(
        np.concatenate([w_hi.T, w_lo.T], axis=1)
    )  # [D, 2E] bf16

    x_hi = x.astype(bf16)
    x_lo = (x - x_hi.astype(np.float32)).astype(bf16)

    in_maps = []
    for c in range(N_CORES):
        rows = slice(c * R, (c + 1) * R)
        xs_c = np.ascontiguousarray(
            np.stack([x_hi[rows].T, x_lo[rows].T], axis=1)
        )  # [D, 2, R]
        in_maps.append({"xsT": xs_c, "wS": wS_np})

    res = run_bass_kernel_spmd(
        nc, in_maps, list(range(N_CORES)), trace=TRACE, tmpdir=TMPDIR
    )
    LAST_RESULT = res
    results = res.results

    idx = np.concatenate([results[c]["idx_out"] for c in range(N_CORES)], axis=0)
    idx = idx.astype(np.int32)  # values 0..63; uint32 -> int32 exact
    tw = np.concatenate([results[c]["w_out"] for c in range(N_CORES)], axis=0)
    ssum = np.stack([results[c]["ssum_out"][0] for c in range(N_CORES)])  # [8, E]

    # host-side tiny reductions for the aux loss
    cores_per_batch = N_CORES // B  # 2
    mean_scores = np.zeros((B, E), np.float32)
    for b in range(B):
        mean_scores[b] = (
            ssum[b * cores_per_batch : (b + 1) * cores_per_batch].sum(axis=0) / S
        )
    idx_b = idx.reshape(B, S * TOP_K)
    ce = np.zeros((B, E), np.float32)
    for b in range(B):
        ce[b] = np.bincount(idx_b[b], minlength=E).astype(np.float32)
    ce /= S * TOP_K / E
    aux_loss = np.float32((ce * mean_scores).sum(axis=1).mean() * ALPHA)

    return idx, tw, aux_loss


# revision 18
# speedup vs baseline: 1.5066x; 1.1704x over previous
"""MoE gate (top-2 of 64 experts) Trainium2 Bass kernel.

Problem: hidden_states [4, 4096, 2048] f32, gate weight [64, 2048] f32.
  logits = x @ W.T            [16384, 64]
  scores = softmax(logits)
  topk_w, topk_i = top_k(scores, 2); topk_w normalized by their sum
  aux_loss from per-batch expert counts (ce) and mean scores.

Sharding: data-parallel over batch*seq. 16384 rows -> 2048 rows/core on 8
cores; the [64, 2048] gate weight is replicated (passed pre-transposed).
x is passed per-core pre-transposed ([D, rows] layout) so the contraction
dim D lands on SBUF partitions with fully-contiguous DMA lines.

Per core device program (Tile framework):
  4 stages x 512 rows. Per stage: 16 k-chunk DMAs [128, 512] feed
  fp32 matmuls accumulating logits into one PSUM tile [128, 4*64]
  (4 row-tiles of 128 rows side by side). Stats: DVE max8/max_index give
  top-2 values+indices per row; ACT exp(logits - max) with accum_out gives
  e and its row-sum; softmax column-sums for the aux loss accumulate on the
  PE via a [128,1] x [128,64] matmul with 1/rowsum as the stationary.
  ce and the final scalar aux loss are reduced on host from returned
  per-core indices and score sums (tiny [8,64] + [16384,2] tensors).
"""

import numpy as np
from contextlib import ExitStack

import concourse.bass as bass
import concourse.tile as tile
from concourse import bacc, mybir
from concourse.bass_utils import run_bass_kernel_spmd

# problem constants (hardcoded per harness contract)
B, S, D, E = 4, 4096, 2048, 64
TOP_K = 2
ALPHA = 0.01
N_CORES = 8
R = (B * S) // N_CORES  # 2048 rows per core
KC = D // 128           # 16 contraction chunks
STAGES = 4
SR = R // STAGES        # 512 rows per stage
JT = SR // 128          # 4 row-tiles per stage

F32 = mybir.dt.float32
U32 = mybir.dt.uint32


def build_moe_gate_kernel():
    nc = bacc.Bacc("TRN2", target_bir_lowering=False, debug=False)

    BF16 = mybir.dt.bfloat16
    # hi/lo bf16 split packed per d-row: [D, 2, R] -> one 1MB DMA per k-chunk
    # with 8KB contiguous partition lines
    xsT = nc.dram_tensor("xsT", [D, 2, R], BF16, kind="ExternalInput").ap()
    # stacked gate weight in SBUF layout [128, KC*2E] (host pre-arranged)
    wS = nc.dram_tensor("wS", [128, KC * 2 * E], BF16, kind="ExternalInput").ap()
    # partition-major packed outputs: [128, STAGES, JT, 2]; host re-permutes
    idx_out = nc.dram_tensor(
        "idx_out", [128, STAGES * JT * TOP_K], U32, kind="ExternalOutput"
    ).ap()
    w_out = nc.dram_tensor(
        "w_out", [128, STAGES * JT * TOP_K], F32, kind="ExternalOutput"
    ).ap()
    ssum_out = nc.dram_tensor("ssum_out", [1, E], F32, kind="ExternalOutput").ap()

    F32R = mybir.dt.float32r

    with tile.TileContext(nc) as tc, ExitStack() as ctx:
        wpool = ctx.enter_context(tc.tile_pool(name="w", bufs=1))
        xpool = ctx.enter_context(tc.tile_pool(name="x", bufs=KC))
        # PSUM banks: 2 lgT (raw [64,512] logits) + 2 lg2 (transposed) + 1 ssum
        lgtpool = ctx.enter_context(tc.tile_pool(name="lgt", bufs=2, space="PSUM"))
        lg2pool = ctx.enter_context(tc.tile_pool(name="lg2", bufs=2, space="PSUM"))
        sspool = ctx.enter_context(tc.tile_pool(name="ss", bufs=1, space="PSUM"))
        spool = ctx.enter_context(tc.tile_pool(name="st", bufs=2))
        epool = ctx.enter_context(tc.tile_pool(name="e", bufs=2 * JT))

        # stacked gate weight, chunk k at [:, k*2E:(k+1)*2E] (bf16, FWL-able)
        wt_sb = wpool.tile([128, KC * 2 * E], BF16)
        nc.sync.dma_start(wt_sb[:], wS)

        # identity for the PE transpose-back of full [128, 128] blocks
        ident_dram = nc.inline_tensor(np.eye(128, dtype=np.float32), name="ident128")
        ident = wpool.tile([128, 128], F32)
        nc.sync.dma_start(ident[:], ident_dram.ap())

        # output collect tiles: one contiguous DMA each at kernel end
        idx_all = wpool.tile([128, STAGES * JT * 8], U32)
        wq_all = wpool.tile([128, STAGES * JT * TOP_K], F32)

        # whole-kernel softmax column-sum accumulator; allocated full-height
        # so the bank isn't shared with (and zeroed under) the logits tiles
        ssum_t = sspool.tile([128, E], F32)
        ssum = ssum_t[0:1, :]

        # all 16 k-chunk DMAs issued up front: 1MB each, 8KB/partition lines;
        # the full split x (16MB) resides in SBUF
        xks = []
        for k in range(KC):
            xk = xpool.tile([128, 2 * R], BF16, tag="xk", name=f"xk_{k}")
            nc.sync.dma_start(xk[:], xsT[k * 128 : (k + 1) * 128, :, :])
            xks.append(xk)

        def emit_mm(s):
            # logits.T for this stage via 2x2 bf16 split: stationary is the
            # stacked [Whi | Wlo] chunk (128 cols, one FWL load per k); the
            # hi and lo row-streams go through it back-to-back at N=512.
            # PSUM [128, 512]: rows 0:64 = Whi^T x*, rows 64:128 = Wlo^T x*.
            lgT = lgtpool.tile([128, SR], F32, tag="lgT", name=f"lgT{s}")
            for k in range(KC):
                stk = wt_sb[:, k * 2 * E : (k + 1) * 2 * E]
                xh = xks[k][:, s * SR : (s + 1) * SR]
                xl = xks[k][:, R + s * SR : R + (s + 1) * SR]
                nc.tensor.matmul(
                    lgT[:], lhsT=stk, rhs=xh, start=(k == 0), stop=False
                )
                nc.tensor.matmul(
                    lgT[:], lhsT=stk, rhs=xl, start=False, stop=(k == KC - 1)
                )
            return lgT

        def emit_stats(s, lgT):
            # PSUM -> SBUF; the Whi (rows 0:64) and Wlo (rows 64:128) halves
            # are summed for free by PSUM accumulation across the two
            # transposes-back to [128 rows, 64 experts]
            lgsb = spool.tile([128, SR], F32, tag="lgsb", name=f"lgsb{s}")
            nc.vector.tensor_copy(lgsb[:], lgT[:])
            # transpose whole [128, 128] blocks: row-tile j's transposed block
            # has hi-logits in cols 0:64 and lo-logits in cols 64:128
            lg2 = lg2pool.tile([128, JT * 128], F32, tag="lg2", name=f"lg2_{s}")
            for j in range(JT):
                nc.tensor.matmul(
                    lg2[:, j * 128 : (j + 1) * 128],
                    lhsT=lgsb[:, j * 128 : (j + 1) * 128],
                    rhs=ident[:],
                    is_transpose=True,
                    start=True,
                    stop=True,
                )
            lg2sb = spool.tile([128, JT * 128], F32, tag="lg2sb", name=f"lg2sb{s}")
            nc.vector.tensor_copy(lg2sb[:], lg2[:])
            # hi + lo halves summed in one strided add -> [128, JT*64] logits
            h3 = lg2sb.rearrange("p (j two e) -> p j two e", two=2, e=E)
            lgs = spool.tile([128, JT * E], F32, tag="lgs", name=f"lgs{s}")
            l3 = lgs.rearrange("p (j e) -> p j e", e=E)
            nc.vector.tensor_add(l3[:, :, :], h3[:, :, 0, :], h3[:, :, 1, :])

            vals8 = spool.tile([128, JT * 8], F32, tag="v8", name=f"v8_{s}")
            idx8 = idx_all[:, s * JT * 8 : (s + 1) * JT * 8]
            for j in range(JT):
                nc.vector.max(vals8[:, j * 8 : (j + 1) * 8], lgs[:, j * E : (j + 1) * E])
                nc.vector.max_index(
                    idx8[:, j * 8 : (j + 1) * 8],
                    vals8[:, j * 8 : (j + 1) * 8],
                    lgs[:, j * E : (j + 1) * E],
                )

            v3 = vals8.rearrange("p (j c) -> p j c", c=8)
            l1 = v3[:, :, 0:1]  # [128, JT, 1] top-1 logit
            l2 = v3[:, :, 1:2]  # top-2 logit

            negm1 = spool.tile([128, JT], F32, tag="nm", name=f"nm_{s}")
            nc.vector.tensor_scalar_mul(negm1[:], l1, -1.0)
            d21 = spool.tile([128, JT], F32, tag="d21", name=f"d21_{s}")
            nc.vector.tensor_sub(d21[:], l2, l1)

            denom = spool.tile([128, JT], F32, tag="dn", name=f"dn_{s}")
            ev = []
            for j in range(JT):
                e_j = epool.tile([128, E], F32, tag="ev", name=f"ev_{s}_{j}")
                nc.scalar.activation(
                    e_j[:],
                    lgs[:, j * E : (j + 1) * E],
                    mybir.ActivationFunctionType.Exp,
                    bias=negm1[:, j : j + 1],
                    scale=1.0,
                    accum_out=denom[:, j : j + 1],
                )
                ev.append(e_j)

            recip = spool.tile([128, JT], F32, tag="rc", name=f"rc_{s}")
            nc.vector.reciprocal(recip[:], denom[:])

            # e2 = exp(l2 - l1); w1 = 1/(1+e2); w2 = e2 * w1
            e2 = spool.tile([128, JT], F32, tag="e2", name=f"e2_{s}")
            nc.scalar.activation(e2[:], d21[:], mybir.ActivationFunctionType.Exp)
            den2 = spool.tile([128, JT], F32, tag="dn2", name=f"dn2_{s}")
            nc.vector.tensor_scalar_add(den2[:], e2[:], 1.0)
            wq3 = wq_all[:, s * JT * 2 : (s + 1) * JT * 2].rearrange(
                "p (j c) -> p j c", c=2
            )
            nc.vector.reciprocal(wq3[:, :, 0:1], den2[:])
            nc.vector.tensor_mul(wq3[:, :, 1:2], e2[:], wq3[:, :, 0:1])

            # scores column-sum: ssum += recip_j^T @ e_j  (over all stages)
            for j in range(JT):
                nc.tensor.matmul(
                    ssum[:],
                    lhsT=recip[:, j : j + 1],
                    rhs=ev[j][:],
                    start=(s == 0 and j == 0),
                    stop=(s == STAGES - 1 and j == JT - 1),
                )


        # software pipeline: stage s matmuls run while stage s-1 stats drain,
        # keeping the PE stream dense across stage boundaries
        lgT_prev = emit_mm(0)
        for s in range(1, STAGES):
            lgT_cur = emit_mm(s)
            emit_stats(s - 1, lgT_prev)
            lgT_prev = lgT_cur
        emit_stats(STAGES - 1, lgT_prev)

        # final output DMAs: contiguous 128B partition lines
        i4 = idx_all.rearrange("p (s j c) -> p s j c", s=STAGES, c=8)
        nc.sync.dma_start(
            idx_out.rearrange("p (s j c) -> p s j c", s=STAGES, c=TOP_K),
            i4[:, :, :, 0:TOP_K],
        )
        nc.sync.dma_start(w_out, wq_all[:])
        ssum_sb = spool.tile([1, E], F32)
        nc.vector.tensor_copy(ssum_sb[:], ssum[:])
        nc.sync.dma_start(ssum_out, ssum_sb[:])

    nc.compile()
    return nc


_NC_CACHE = None

# test-harness knobs (harness never touches these; kernel() defaults are fine)
TRACE = False
TMPDIR = None
LAST_RESULT = None


def _get_nc():
    global _NC_CACHE
    if _NC_CACHE is None:
        _NC_CACHE = build_moe_gate_kernel()
    return _NC_CACHE


def kernel(hidden_states: np.ndarray, weight: np.ndarray):
    global LAST_RESULT
    import ml_dtypes

    bf16 = ml_dtypes.bfloat16
    nc = _get_nc()
    x = np.asarray(hidden_states, dtype=np.float32).reshape(B * S, D)
    w = np.asarray(weight, dtype=np.float32)

    # 2x2 bf16 split: v = hi + lo with hi = bf16(v), lo = bf16(v - hi);
    # [Whi | Wlo] stacked so one 128-col stationary serves both terms
    w_hi = w.astype(bf16)
    w_lo = (w - w_hi.astype(np.float32)).astype(bf16)
    wcat = np.concatenate([w_hi.T, w_lo.T], axis=1)  # [D, 2E]
    # SBUF layout: [128, KC*2E] with chunk k at cols [k*2E:(k+1)*2E]
    wS_np = np.ascontiguousarray(
        wcat.reshape(KC, 128, 2 * E).transpose(1, 0, 2).reshape(128, KC * 2 * E)
    )

    x_hi = x.astype(bf16)
    x_lo = (x - x_hi.astype(np.float32)).astype(bf16)

    in_maps = []
    for c in range(N_CORES):
        rows = slice(c * R, (c + 1) * R)
        xs_c = np.ascontiguousarray(
            np.stack([x_hi[rows].T, x_lo[rows].T], axis=1)
        )  # [D, 2, R]
        in_maps.append({"xsT": xs_c, "wS": wS_np})

    res = run_bass_kernel_spmd(
        nc, in_maps, list(range(N_CORES)), trace=TRACE, tmpdir=TMPDIR
    )
    LAST_RESULT = res
    results = res.results

    def unpack(a):
        # [128, STAGES*JT*2] -> rows (s*SR + j*128 + p), cols c
        return (
            a.reshape(128, STAGES, JT, TOP_K)
            .transpose(1, 2, 0, 3)
            .reshape(R, TOP_K)
        )

    idx = np.concatenate(
        [unpack(results[c]["idx_out"]) for c in range(N_CORES)], axis=0
    )
    idx = idx.astype(np.int32)  # values 0..63; uint32 -> int32 exact
    tw = np.concatenate(
        [unpack(results[c]["w_out"]) for c in range(N_CORES)], axis=0
    )
    ssum = np.stack([results[c]["ssum_out"][0] for c in range(N_CORES)])  # [8, E]

    # host-side tiny reductions for the aux loss
    cores_per_batch = N_CORES // B  # 2
    mean_scores = np.zeros((B, E), np.float32)
    for b in range(B):
        mean_scores[b] = (
            ssum[b * cores_per_batch : (b + 1) * cores_per_batch].sum(axis=0) / S
        )
    idx_b = idx.reshape(B, S * TOP_K)
    ce = np.zeros((B, E), np.float32)
    for b in range(B):
        ce[b] = np.bincount(idx_b[b], minlength=E).astype(np.float32)
    ce /= S * TOP_K / E
    aux_loss = np.float32((ce * mean_scores).sum(axis=1).mean() * ALPHA)

    return idx, tw, aux_loss


# revision 19
# speedup vs baseline: 1.9476x; 1.2928x over previous
"""MoE gate (top-2 of 64 experts) Trainium2 Bass kernel.

Problem: hidden_states [4, 4096, 2048] f32, gate weight [64, 2048] f32.
  logits = x @ W.T            [16384, 64]
  scores = softmax(logits)
  topk_w, topk_i = top_k(scores, 2); topk_w normalized by their sum
  aux_loss from per-batch expert counts (ce) and mean scores.

Sharding: data-parallel over batch*seq. 16384 rows -> 2048 rows/core on 8
cores; the [64, 2048] gate weight is replicated (passed pre-transposed).
x is passed per-core pre-transposed ([D, rows] layout) so the contraction
dim D lands on SBUF partitions with fully-contiguous DMA lines.

Per core device program (Tile framework):
  4 stages x 512 rows. Per stage: 16 k-chunk DMAs [128, 512] feed
  fp32 matmuls accumulating logits into one PSUM tile [128, 4*64]
  (4 row-tiles of 128 rows side by side). Stats: DVE max8/max_index give
  top-2 values+indices per row; ACT exp(logits - max) with accum_out gives
  e and its row-sum; softmax column-sums for the aux loss accumulate on the
  PE via a [128,1] x [128,64] matmul with 1/rowsum as the stationary.
  ce and the final scalar aux loss are reduced on host from returned
  per-core indices and score sums (tiny [8,64] + [16384,2] tensors).
"""

import numpy as np
from contextlib import ExitStack

import concourse.bass as bass
import concourse.tile as tile
from concourse import bacc, mybir
from concourse.bass_utils import run_bass_kernel_spmd

# problem constants (hardcoded per harness contract)
B, S, D, E = 4, 4096, 2048, 64
TOP_K = 2
ALPHA = 0.01
N_CORES = 8
R = (B * S) // N_CORES  # 2048 rows per core
KC = D // 128           # 16 contraction chunks
STAGES = 4
SR = R // STAGES        # 512 rows per stage
JT = SR // 128          # 4 row-tiles per stage

F32 = mybir.dt.float32
U32 = mybir.dt.uint32


def build_moe_gate_kernel():
    nc = bacc.Bacc("TRN2", target_bir_lowering=False, debug=False)

    BF16 = mybir.dt.bfloat16
    # hi/lo bf16 split, packed [D, 2 dma-halves, 2 hi/lo, R/2]: 32 DMAs with
    # 4KB contiguous partition lines whose arrival order matches the PE's
    # stage-by-stage consumption order
    xsT = nc.dram_tensor("xsT", [D, 2, 2, R // 2], BF16, kind="ExternalInput").ap()
    # stacked gate weight in SBUF layout [128, KC*2E] (host pre-arranged)
    wS = nc.dram_tensor("wS", [128, KC * 2 * E], BF16, kind="ExternalInput").ap()
    # partition-major packed outputs: [128, STAGES, JT, 2]; host re-permutes
    idx_out = nc.dram_tensor(
        "idx_out", [128, STAGES * JT * TOP_K], U32, kind="ExternalOutput"
    ).ap()
    w_out = nc.dram_tensor(
        "w_out", [128, STAGES * JT * TOP_K], F32, kind="ExternalOutput"
    ).ap()
    ssum_out = nc.dram_tensor("ssum_out", [1, E], F32, kind="ExternalOutput").ap()

    F32R = mybir.dt.float32r

    with tile.TileContext(nc) as tc, ExitStack() as ctx:
        wpool = ctx.enter_context(tc.tile_pool(name="w", bufs=1))
        xpool = ctx.enter_context(tc.tile_pool(name="x", bufs=2 * KC))
        # PSUM banks: 2 lgT (raw [64,512] logits) + 2 lg2 (transposed) + 1 ssum
        lgtpool = ctx.enter_context(tc.tile_pool(name="lgt", bufs=2, space="PSUM"))
        lg2pool = ctx.enter_context(tc.tile_pool(name="lg2", bufs=2, space="PSUM"))
        sspool = ctx.enter_context(tc.tile_pool(name="ss", bufs=1, space="PSUM"))
        spool = ctx.enter_context(tc.tile_pool(name="st", bufs=2))
        epool = ctx.enter_context(tc.tile_pool(name="e", bufs=2 * JT))

        # stacked gate weight, chunk k at [:, k*2E:(k+1)*2E] (bf16, FWL-able)
        wt_sb = wpool.tile([128, KC * 2 * E], BF16)
        nc.sync.dma_start(wt_sb[:], wS)

        # identity for the PE transpose-back of full [128, 128] blocks
        ident_dram = nc.inline_tensor(np.eye(128, dtype=np.float32), name="ident128")
        ident = wpool.tile([128, 128], F32)
        nc.sync.dma_start(ident[:], ident_dram.ap())

        # output collect tiles: one contiguous DMA each at kernel end
        idx_all = wpool.tile([128, STAGES * JT * 8], U32)
        wq_all = wpool.tile([128, STAGES * JT * TOP_K], F32)

        # whole-kernel softmax column-sum accumulator; allocated full-height
        # so the bank isn't shared with (and zeroed under) the logits tiles
        ssum_t = sspool.tile([128, E], F32)
        ssum = ssum_t[0:1, :]

        # all 32 DMAs issued up front in consumption order: 512KB each,
        # 4KB/partition lines; the full split x (16MB) resides in SBUF
        RD = R // 2
        xks = []
        for ds in range(2):
            for k in range(KC):
                xk = xpool.tile(
                    [128, 2 * RD], BF16, tag="xk", name=f"xk_{ds}_{k}"
                )
                nc.sync.dma_start(
                    xk[:], xsT[k * 128 : (k + 1) * 128, ds, :, :]
                )
                xks.append(xk)

        def emit_mm(s):
            # logits.T for this stage via 2x2 bf16 split: stationary is the
            # stacked [Whi | Wlo] chunk (128 cols, one FWL load per k); the
            # hi and lo row-streams go through it back-to-back at N=512.
            # PSUM [128, 512]: rows 0:64 = Whi^T x*, rows 64:128 = Wlo^T x*.
            lgT = lgtpool.tile([128, SR], F32, tag="lgT", name=f"lgT{s}")
            ds, ro = s // 2, (s % 2) * SR
            for k in range(KC):
                stk = wt_sb[:, k * 2 * E : (k + 1) * 2 * E]
                xh = xks[ds * KC + k][:, ro : ro + SR]
                xl = xks[ds * KC + k][:, RD + ro : RD + ro + SR]
                nc.tensor.matmul(
                    lgT[:], lhsT=stk, rhs=xh, start=(k == 0), stop=False
                )
                nc.tensor.matmul(
                    lgT[:], lhsT=stk, rhs=xl, start=False, stop=(k == KC - 1)
                )
            return lgT

        def emit_stats(s, lgT):
            # PSUM -> SBUF; the Whi (rows 0:64) and Wlo (rows 64:128) halves
            # are summed for free by PSUM accumulation across the two
            # transposes-back to [128 rows, 64 experts]
            lgsb = spool.tile([128, SR], F32, tag="lgsb", name=f"lgsb{s}")
            nc.vector.tensor_copy(lgsb[:], lgT[:])
            # transpose whole [128, 128] blocks: row-tile j's transposed block
            # has hi-logits in cols 0:64 and lo-logits in cols 64:128
            lg2 = lg2pool.tile([128, JT * 128], F32, tag="lg2", name=f"lg2_{s}")
            for j in range(JT):
                nc.tensor.matmul(
                    lg2[:, j * 128 : (j + 1) * 128],
                    lhsT=lgsb[:, j * 128 : (j + 1) * 128],
                    rhs=ident[:],
                    is_transpose=True,
                    start=True,
                    stop=True,
                )
            lg2sb = spool.tile([128, JT * 128], F32, tag="lg2sb", name=f"lg2sb{s}")
            nc.vector.tensor_copy(lg2sb[:], lg2[:])
            # hi + lo halves summed in one strided add -> [128, JT*64] logits
            h3 = lg2sb.rearrange("p (j two e) -> p j two e", two=2, e=E)
            lgs = spool.tile([128, JT * E], F32, tag="lgs", name=f"lgs{s}")
            l3 = lgs.rearrange("p (j e) -> p j e", e=E)
            nc.vector.tensor_add(l3[:, :, :], h3[:, :, 0, :], h3[:, :, 1, :])

            vals8 = spool.tile([128, JT * 8], F32, tag="v8", name=f"v8_{s}")
            idx8 = idx_all[:, s * JT * 8 : (s + 1) * JT * 8]
            for j in range(JT):
                nc.vector.max(vals8[:, j * 8 : (j + 1) * 8], lgs[:, j * E : (j + 1) * E])
                nc.vector.max_index(
                    idx8[:, j * 8 : (j + 1) * 8],
                    vals8[:, j * 8 : (j + 1) * 8],
                    lgs[:, j * E : (j + 1) * E],
                )

            v3 = vals8.rearrange("p (j c) -> p j c", c=8)
            l1 = v3[:, :, 0:1]  # [128, JT, 1] top-1 logit
            l2 = v3[:, :, 1:2]  # top-2 logit

            negm1 = spool.tile([128, JT], F32, tag="nm", name=f"nm_{s}")
            nc.vector.tensor_scalar_mul(negm1[:], l1, -1.0)
            d21 = spool.tile([128, JT], F32, tag="d21", name=f"d21_{s}")
            nc.vector.tensor_sub(d21[:], l2, l1)

            denom = spool.tile([128, JT], F32, tag="dn", name=f"dn_{s}")
            ev = []
            for j in range(JT):
                e_j = epool.tile([128, E], F32, tag="ev", name=f"ev_{s}_{j}")
                nc.scalar.activation(
                    e_j[:],
                    lgs[:, j * E : (j + 1) * E],
                    mybir.ActivationFunctionType.Exp,
                    bias=negm1[:, j : j + 1],
                    scale=1.0,
                    accum_out=denom[:, j : j + 1],
                )
                ev.append(e_j)

            recip = spool.tile([128, JT], F32, tag="rc", name=f"rc_{s}")
            nc.vector.reciprocal(recip[:], denom[:])

            # e2 = exp(l2 - l1); w1 = 1/(1+e2); w2 = e2 * w1
            e2 = spool.tile([128, JT], F32, tag="e2", name=f"e2_{s}")
            nc.scalar.activation(e2[:], d21[:], mybir.ActivationFunctionType.Exp)
            den2 = spool.tile([128, JT], F32, tag="dn2", name=f"dn2_{s}")
            nc.vector.tensor_scalar_add(den2[:], e2[:], 1.0)
            wq3 = wq_all[:, s * JT * 2 : (s + 1) * JT * 2].rearrange(
                "p (j c) -> p j c", c=2
            )
            nc.vector.reciprocal(wq3[:, :, 0:1], den2[:])
            nc.vector.tensor_mul(wq3[:, :, 1:2], e2[:], wq3[:, :, 0:1])

            # scores column-sum: ssum += recip_j^T @ e_j  (over all stages)
            for j in range(JT):
                nc.tensor.matmul(
                    ssum[:],
                    lhsT=recip[:, j : j + 1],
                    rhs=ev[j][:],
                    start=(s == 0 and j == 0),
                    stop=(s == STAGES - 1 and j == JT - 1),
                )


        # software pipeline: stage s matmuls run while stage s-1 stats drain,
        # keeping the PE stream dense across stage boundaries
        lgT_prev = emit_mm(0)
        for s in range(1, STAGES):
            lgT_cur = emit_mm(s)
            emit_stats(s - 1, lgT_prev)
            lgT_prev = lgT_cur
        emit_stats(STAGES - 1, lgT_prev)

        # final output DMAs: contiguous 128B partition lines
        i4 = idx_all.rearrange("p (s j c) -> p s j c", s=STAGES, c=8)
        nc.sync.dma_start(
            idx_out.rearrange("p (s j c) -> p s j c", s=STAGES, c=TOP_K),
            i4[:, :, :, 0:TOP_K],
        )
        nc.sync.dma_start(w_out, wq_all[:])
        ssum_sb = spool.tile([1, E], F32)
        nc.vector.tensor_copy(ssum_sb[:], ssum[:])
        nc.sync.dma_start(ssum_out, ssum_sb[:])

    nc.compile()
    return nc


_NC_CACHE = None

# test-harness knobs (harness never touches these; kernel() defaults are fine)
TRACE = False
TMPDIR = None
LAST_RESULT = None


def _get_nc():
    global _NC_CACHE
    if _NC_CACHE is None:
        _NC_CACHE = build_moe_gate_kernel()
    return _NC_CACHE


def kernel(hidden_states: np.ndarray, weight: np.ndarray):
    global LAST_RESULT
    import ml_dtypes

    bf16 = ml_dtypes.bfloat16
    nc = _get_nc()
    x = np.asarray(hidden_states, dtype=np.float32).reshape(B * S, D)
    w = np.asarray(weight, dtype=np.float32)

    # 2x2 bf16 split: v = hi + lo with hi = bf16(v), lo = bf16(v - hi);
    # [Whi | Wlo] stacked so one 128-col stationary serves both terms
    w_hi = w.astype(bf16)
    w_lo = (w - w_hi.astype(np.float32)).astype(bf16)
    wcat = np.concatenate([w_hi.T, w_lo.T], axis=1)  # [D, 2E]
    # SBUF layout: [128, KC*2E] with chunk k at cols [k*2E:(k+1)*2E]
    wS_np = np.ascontiguousarray(
        wcat.reshape(KC, 128, 2 * E).transpose(1, 0, 2).reshape(128, KC * 2 * E)
    )

    x_hi = x.astype(bf16)
    x_lo = (x - x_hi.astype(np.float32)).astype(bf16)

    in_maps = []
    for c in range(N_CORES):
        rows = slice(c * R, (c + 1) * R)
        xs_c = np.ascontiguousarray(
            np.stack(
                [
                    x_hi[rows].T.reshape(D, 2, R // 2),
                    x_lo[rows].T.reshape(D, 2, R // 2),
                ],
                axis=2,
            )
        )  # [D, 2, 2, R/2]
        in_maps.append({"xsT": xs_c, "wS": wS_np})

    res = run_bass_kernel_spmd(
        nc, in_maps, list(range(N_CORES)), trace=TRACE, tmpdir=TMPDIR
    )
    LAST_RESULT = res
    results = res.results

    def unpack(a):
        # [128, STAGES*JT*2] -> rows (s*SR + j*128 + p), cols c
        return (
            a.reshape(128, STAGES, JT, TOP_K)
            .transpose(1, 2, 0, 3)
            .reshape(R, TOP_K)
        )

    idx = np.concatenate(
        [unpack(results[c]["idx_out"]) for c in range(N_CORES)], axis=0
    )
    idx = idx.astype(np.int32)  # values 0..63; uint32 -> int32 exact
    tw = np.concatenate(
        [unpack(results[c]["w_out"]) for c in range(N_CORES)], axis=0
    )
    ssum = np.stack([results[c]["ssum_out"][0] for c in range(N_CORES)])  # [8, E]

    # host-side tiny reductions for the aux loss
    cores_per_batch = N_CORES // B  # 2
    mean_scores = np.zeros((B, E), np.float32)
    for b in range(B):
        mean_scores[b] = (
            ssum[b * cores_per_batch : (b + 1) * cores_per_batch].sum(axis=0) / S
        )
    idx_b = idx.reshape(B, S * TOP_K)
    ce = np.zeros((B, E), np.float32)
    for b in range(B):
        ce[b] = np.bincount(idx_b[b], minlength=E).astype(np.float32)
    ce /= S * TOP_K / E
    aux_loss = np.float32((ce * mean_scores).sum(axis=1).mean() * ALPHA)

    return idx, tw, aux_loss


# revision 21
# speedup vs baseline: 1.9526x; 1.0025x over previous
"""MoE gate (top-2 of 64 experts) Trainium2 Bass kernel.

Problem: hidden_states [4, 4096, 2048] f32, gate weight [64, 2048] f32.
  logits = x @ W.T            [16384, 64]
  scores = softmax(logits)
  topk_w, topk_i = top_k(scores, 2); topk_w normalized by their sum
  aux_loss from per-batch expert counts (ce) and mean scores.

Sharding: data-parallel over batch*seq. 16384 rows -> 2048 rows/core on 8
cores; the [64, 2048] gate weight is replicated (passed pre-transposed).
x is passed per-core pre-transposed ([D, rows] layout) so the contraction
dim D lands on SBUF partitions with fully-contiguous DMA lines.

Per core device program (Tile framework):
  4 stages x 512 rows. Per stage: 16 k-chunk DMAs [128, 512] feed
  fp32 matmuls accumulating logits into one PSUM tile [128, 4*64]
  (4 row-tiles of 128 rows side by side). Stats: DVE max8/max_index give
  top-2 values+indices per row; ACT exp(logits - max) with accum_out gives
  e and its row-sum; softmax column-sums for the aux loss accumulate on the
  PE via a [128,1] x [128,64] matmul with 1/rowsum as the stationary.
  ce and the final scalar aux loss are reduced on host from returned
  per-core indices and score sums (tiny [8,64] + [16384,2] tensors).
"""

import numpy as np
from contextlib import ExitStack

import concourse.bass as bass
import concourse.tile as tile
from concourse import bacc, mybir
from concourse.bass_utils import run_bass_kernel_spmd

# problem constants (hardcoded per harness contract)
B, S, D, E = 4, 4096, 2048, 64
TOP_K = 2
ALPHA = 0.01
N_CORES = 8
R = (B * S) // N_CORES  # 2048 rows per core
KC = D // 128           # 16 contraction chunks
STAGES = 4
SR = R // STAGES        # 512 rows per stage
JT = SR // 128          # 4 row-tiles per stage

F32 = mybir.dt.float32
U32 = mybir.dt.uint32


def build_moe_gate_kernel():
    nc = bacc.Bacc("TRN2", target_bir_lowering=False, debug=False)

    BF16 = mybir.dt.bfloat16
    # hi/lo bf16 split, packed [D, 2 dma-halves, 2 hi/lo, R/2]: 32 DMAs with
    # 4KB contiguous partition lines whose arrival order matches the PE's
    # stage-by-stage consumption order
    xsT = nc.dram_tensor("xsT", [D, 2, 2, R // 2], BF16, kind="ExternalInput").ap()
    # stacked gate weight in SBUF layout [128, KC*2E] (host pre-arranged)
    wS = nc.dram_tensor("wS", [128, KC * 2 * E], BF16, kind="ExternalInput").ap()
    # partition-major packed outputs: [128, STAGES, JT, 2]; host re-permutes
    idx_out = nc.dram_tensor(
        "idx_out", [128, STAGES * JT * TOP_K], U32, kind="ExternalOutput"
    ).ap()
    w_out = nc.dram_tensor(
        "w_out", [128, STAGES * JT * TOP_K], F32, kind="ExternalOutput"
    ).ap()
    ssum_out = nc.dram_tensor("ssum_out", [1, E], F32, kind="ExternalOutput").ap()

    F32R = mybir.dt.float32r

    with tile.TileContext(nc) as tc, ExitStack() as ctx:
        wpool = ctx.enter_context(tc.tile_pool(name="w", bufs=1))
        xpool = ctx.enter_context(tc.tile_pool(name="x", bufs=2 * KC))
        # PSUM banks: 2 lgT (raw [64,512] logits) + 2 lg2 (transposed) + 1 ssum
        lgtpool = ctx.enter_context(tc.tile_pool(name="lgt", bufs=3, space="PSUM"))
        lg2pool = ctx.enter_context(tc.tile_pool(name="lg2", bufs=2, space="PSUM"))
        sspool = ctx.enter_context(tc.tile_pool(name="ss", bufs=1, space="PSUM"))
        spool = ctx.enter_context(tc.tile_pool(name="st", bufs=2))
        epool = ctx.enter_context(tc.tile_pool(name="e", bufs=2 * JT))

        # stacked gate weight, chunk k at [:, k*2E:(k+1)*2E] (bf16, FWL-able)
        wt_sb = wpool.tile([128, KC * 2 * E], BF16)
        nc.sync.dma_start(wt_sb[:], wS)

        # identity for the PE transpose-back of full [128, 128] blocks
        ident_dram = nc.inline_tensor(np.eye(128, dtype=np.float32), name="ident128")
        ident = wpool.tile([128, 128], F32)
        nc.sync.dma_start(ident[:], ident_dram.ap())

        # output collect tiles: one contiguous DMA each at kernel end
        idx_all = wpool.tile([128, STAGES * JT * 8], U32)
        wq_all = wpool.tile([128, STAGES * JT * TOP_K], F32)

        # whole-kernel softmax column-sum accumulator; allocated full-height
        # so the bank isn't shared with (and zeroed under) the logits tiles
        ssum_t = sspool.tile([128, E], F32)
        ssum = ssum_t[0:1, :]

        # all 32 DMAs issued up front in consumption order: 512KB each,
        # 4KB/partition lines; the full split x (16MB) resides in SBUF
        RD = R // 2
        xks = []
        for ds in range(2):
            for k in range(KC):
                xk = xpool.tile(
                    [128, 2 * RD], BF16, tag="xk", name=f"xk_{ds}_{k}"
                )
                nc.sync.dma_start(
                    xk[:], xsT[k * 128 : (k + 1) * 128, ds, :, :]
                )
                xks.append(xk)

        def emit_mm_half(ds):
            # both stages of this dma-half interleaved per k-chunk, so each
            # arriving 512KB chunk is fully consumed (4 MMs) at once and the
            # last chunk leaves only 4 MMs + one stats chain to drain.
            # 2x2 bf16 split: stationary = stacked [Whi | Wlo] chunk (128
            # cols, one FWL load reused by the hi and lo N=512 streams).
            # PSUM [128, 512]: rows 0:64 = Whi^T x*, rows 64:128 = Wlo^T x*.
            lgTs_pair = [
                lgtpool.tile([128, SR], F32, tag="lgT", name=f"lgT{2 * ds + h}")
                for h in range(2)
            ]
            for k in range(KC):
                stk = wt_sb[:, k * 2 * E : (k + 1) * 2 * E]
                for h in range(2):
                    ro = h * SR
                    xh = xks[ds * KC + k][:, ro : ro + SR]
                    xl = xks[ds * KC + k][:, RD + ro : RD + ro + SR]
                    nc.tensor.matmul(
                        lgTs_pair[h][:],
                        lhsT=stk,
                        rhs=xh,
                        start=(k == 0),
                        stop=False,
                    )
                    nc.tensor.matmul(
                        lgTs_pair[h][:],
                        lhsT=stk,
                        rhs=xl,
                        start=False,
                        stop=(k == KC - 1),
                    )
            return lgTs_pair

        def emit_stats(s, lgT):
            # PSUM -> SBUF; the Whi (rows 0:64) and Wlo (rows 64:128) halves
            # are summed for free by PSUM accumulation across the two
            # transposes-back to [128 rows, 64 experts]
            lgsb = spool.tile([128, SR], F32, tag="lgsb", name=f"lgsb{s}")
            nc.vector.tensor_copy(lgsb[:], lgT[:])
            # transpose whole [128, 128] blocks: row-tile j's transposed block
            # has hi-logits in cols 0:64 and lo-logits in cols 64:128
            lg2 = lg2pool.tile([128, JT * 128], F32, tag="lg2", name=f"lg2_{s}")
            for j in range(JT):
                nc.tensor.matmul(
                    lg2[:, j * 128 : (j + 1) * 128],
                    lhsT=lgsb[:, j * 128 : (j + 1) * 128],
                    rhs=ident[:],
                    is_transpose=True,
                    start=True,
                    stop=True,
                )
            lg2sb = spool.tile([128, JT * 128], F32, tag="lg2sb", name=f"lg2sb{s}")
            nc.vector.tensor_copy(lg2sb[:], lg2[:])
            # hi + lo halves summed in one strided add -> [128, JT*64] logits
            h3 = lg2sb.rearrange("p (j two e) -> p j two e", two=2, e=E)
            lgs = spool.tile([128, JT * E], F32, tag="lgs", name=f"lgs{s}")
            l3 = lgs.rearrange("p (j e) -> p j e", e=E)
            nc.vector.tensor_add(l3[:, :, :], h3[:, :, 0, :], h3[:, :, 1, :])

            vals8 = spool.tile([128, JT * 8], F32, tag="v8", name=f"v8_{s}")
            idx8 = idx_all[:, s * JT * 8 : (s + 1) * JT * 8]
            for j in range(JT):
                nc.vector.max(vals8[:, j * 8 : (j + 1) * 8], lgs[:, j * E : (j + 1) * E])
                nc.vector.max_index(
                    idx8[:, j * 8 : (j + 1) * 8],
                    vals8[:, j * 8 : (j + 1) * 8],
                    lgs[:, j * E : (j + 1) * E],
                )

            v3 = vals8.rearrange("p (j c) -> p j c", c=8)
            l1 = v3[:, :, 0:1]  # [128, JT, 1] top-1 logit
            l2 = v3[:, :, 1:2]  # top-2 logit

            negm1 = spool.tile([128, JT], F32, tag="nm", name=f"nm_{s}")
            nc.vector.tensor_scalar_mul(negm1[:], l1, -1.0)
            d21 = spool.tile([128, JT], F32, tag="d21", name=f"d21_{s}")
            nc.vector.tensor_sub(d21[:], l2, l1)

            denom = spool.tile([128, JT], F32, tag="dn", name=f"dn_{s}")
            ev = []
            for j in range(JT):
                e_j = epool.tile([128, E], F32, tag="ev", name=f"ev_{s}_{j}")
                nc.scalar.activation(
                    e_j[:],
                    lgs[:, j * E : (j + 1) * E],
                    mybir.ActivationFunctionType.Exp,
                    bias=negm1[:, j : j + 1],
                    scale=1.0,
                    accum_out=denom[:, j : j + 1],
                )
                ev.append(e_j)

            recip = spool.tile([128, JT], F32, tag="rc", name=f"rc_{s}")
            nc.vector.reciprocal(recip[:], denom[:])

            # e2 = exp(l2 - l1); w1 = 1/(1+e2); w2 = e2 * w1
            e2 = spool.tile([128, JT], F32, tag="e2", name=f"e2_{s}")
            nc.scalar.activation(e2[:], d21[:], mybir.ActivationFunctionType.Exp)
            den2 = spool.tile([128, JT], F32, tag="dn2", name=f"dn2_{s}")
            nc.vector.tensor_scalar_add(den2[:], e2[:], 1.0)
            wq3 = wq_all[:, s * JT * 2 : (s + 1) * JT * 2].rearrange(
                "p (j c) -> p j c", c=2
            )
            nc.vector.reciprocal(wq3[:, :, 0:1], den2[:])
            nc.vector.tensor_mul(wq3[:, :, 1:2], e2[:], wq3[:, :, 0:1])

            # scores column-sum: ssum += recip_j^T @ e_j  (over all stages)
            for j in range(JT):
                nc.tensor.matmul(
                    ssum[:],
                    lhsT=recip[:, j : j + 1],
                    rhs=ev[j][:],
                    start=(s == 0 and j == 0),
                    stop=(s == STAGES - 1 and j == JT - 1),
                )


        # software pipeline: half 0 (stages 0,1) matmuls, then half 1's
        # matmuls run while half 0's stats drain
        lg01 = emit_mm_half(0)
        emit_stats(0, lg01[0])
        emit_stats(1, lg01[1])
        lg23 = emit_mm_half(1)
        emit_stats(2, lg23[0])
        emit_stats(3, lg23[1])

        # final output DMAs: contiguous 128B partition lines
        i4 = idx_all.rearrange("p (s j c) -> p s j c", s=STAGES, c=8)
        nc.sync.dma_start(
            idx_out.rearrange("p (s j c) -> p s j c", s=STAGES, c=TOP_K),
            i4[:, :, :, 0:TOP_K],
        )
        nc.sync.dma_start(w_out, wq_all[:])
        ssum_sb = spool.tile([1, E], F32)
        nc.vector.tensor_copy(ssum_sb[:], ssum[:])
        nc.sync.dma_start(ssum_out, ssum_sb[:])

    nc.compile()
    return nc


_NC_CACHE = None

# test-harness knobs (harness never touches these; kernel() defaults are fine)
TRACE = False
TMPDIR = None
LAST_RESULT = None


def _get_nc():
    global _NC_CACHE
    if _NC_CACHE is None:
        _NC_CACHE = build_moe_gate_kernel()
    return _NC_CACHE


def kernel(hidden_states: np.ndarray, weight: np.ndarray):
    global LAST_RESULT
    import ml_dtypes

    bf16 = ml_dtypes.bfloat16
    nc = _get_nc()
    x = np.asarray(hidden_states, dtype=np.float32).reshape(B * S, D)
    w = np.asarray(weight, dtype=np.float32)

    # 2x2 bf16 split: v = hi + lo with hi = bf16(v), lo = bf16(v - hi);
    # [Whi | Wlo] stacked so one 128-col stationary serves both terms
    w_hi = w.astype(bf16)
    w_lo = (w - w_hi.astype(np.float32)).astype(bf16)
    wcat = np.concatenate([w_hi.T, w_lo.T], axis=1)  # [D, 2E]
    # SBUF layout: [128, KC*2E] with chunk k at cols [k*2E:(k+1)*2E]
    wS_np = np.ascontiguousarray(
        wcat.reshape(KC, 128, 2 * E).transpose(1, 0, 2).reshape(128, KC * 2 * E)
    )

    x_hi = x.astype(bf16)
    x_lo = (x - x_hi.astype(np.float32)).astype(bf16)

    in_maps = []
    for c in range(N_CORES):
        rows = slice(c * R, (c + 1) * R)
        xs_c = np.ascontiguousarray(
            np.stack(
                [
                    x_hi[rows].T.reshape(D, 2, R // 2),
                    x_lo[rows].T.reshape(D, 2, R // 2),
                ],
                axis=2,
            )
        )  # [D, 2, 2, R/2]
        in_maps.append({"xsT": xs_c, "wS": wS_np})

    res = run_bass_kernel_spmd(
        nc, in_maps, list(range(N_CORES)), trace=TRACE, tmpdir=TMPDIR
    )
    LAST_RESULT = res
    results = res.results

    def unpack(a):
        # [128, STAGES*JT*2] -> rows (s*SR + j*128 + p), cols c
        return (
            a.reshape(128, STAGES, JT, TOP_K)
            .transpose(1, 2, 0, 3)
            .reshape(R, TOP_K)
        )

    idx = np.concatenate(
        [unpack(results[c]["idx_out"]) for c in range(N_CORES)], axis=0
    )
    idx = idx.astype(np.int32)  # values 0..63; uint32 -> int32 exact
    tw = np.concatenate(
        [unpack(results[c]["w_out"]) for c in range(N_CORES)], axis=0
    )
    ssum = np.stack([results[c]["ssum_out"][0] for c in range(N_CORES)])  # [8, E]

    # host-side tiny reductions for the aux loss
    cores_per_batch = N_CORES // B  # 2
    mean_scores = np.zeros((B, E), np.float32)
    for b in range(B):
        mean_scores[b] = (
            ssum[b * cores_per_batch : (b + 1) * cores_per_batch].sum(axis=0) / S
        )
    idx_b = idx.reshape(B, S * TOP_K)
    ce = np.zeros((B, E), np.float32)
    for b in range(B):
        ce[b] = np.bincount(idx_b[b], minlength=E).astype(np.float32)
    ce /= S * TOP_K / E
    aux_loss = np.float32((ce * mean_scores).sum(axis=1).mean() * ALPHA)

    return idx, tw, aux_loss


# revision 22
# speedup vs baseline: 1.9676x; 1.0077x over previous
"""MoE gate (top-2 of 64 experts) Trainium2 Bass kernel.

Problem: hidden_states [4, 4096, 2048] f32, gate weight [64, 2048] f32.
  logits = x @ W.T            [16384, 64]
  scores = softmax(logits)
  topk_w, topk_i = top_k(scores, 2); topk_w normalized by their sum
  aux_loss from per-batch expert counts (ce) and mean scores.

Sharding: data-parallel over batch*seq. 16384 rows -> 2048 rows/core on 8
cores; the [64, 2048] gate weight is replicated (passed pre-transposed).
x is passed per-core pre-transposed ([D, rows] layout) so the contraction
dim D lands on SBUF partitions with fully-contiguous DMA lines.

Per core device program (Tile framework):
  4 stages x 512 rows. Per stage: 16 k-chunk DMAs [128, 512] feed
  fp32 matmuls accumulating logits into one PSUM tile [128, 4*64]
  (4 row-tiles of 128 rows side by side). Stats: DVE max8/max_index give
  top-2 values+indices per row; ACT exp(logits - max) with accum_out gives
  e and its row-sum; softmax column-sums for the aux loss accumulate on the
  PE via a [128,1] x [128,64] matmul with 1/rowsum as the stationary.
  ce and the final scalar aux loss are reduced on host from returned
  per-core indices and score sums (tiny [8,64] + [16384,2] tensors).
"""

import numpy as np
from contextlib import ExitStack

import concourse.bass as bass
import concourse.tile as tile
from concourse import bacc, mybir
from concourse.bass_utils import run_bass_kernel_spmd

# problem constants (hardcoded per harness contract)
B, S, D, E = 4, 4096, 2048, 64
TOP_K = 2
ALPHA = 0.01
N_CORES = 8
R = (B * S) // N_CORES  # 2048 rows per core
KC = D // 128           # 16 contraction chunks
STAGES = 4
SR = R // STAGES        # 512 rows per stage
JT = SR // 128          # 4 row-tiles per stage

F32 = mybir.dt.float32
U32 = mybir.dt.uint32


def build_moe_gate_kernel():
    nc = bacc.Bacc("TRN2", target_bir_lowering=False, debug=False)

    BF16 = mybir.dt.bfloat16
    # hi/lo bf16 split, packed [D, 2 dma-halves, 2 hi/lo, R/2]: 32 DMAs with
    # 4KB contiguous partition lines whose arrival order matches the PE's
    # stage-by-stage consumption order
    xsT = nc.dram_tensor("xsT", [D, 2, 2, R // 2], BF16, kind="ExternalInput").ap()
    # stacked gate weight in SBUF layout [128, KC*2E] (host pre-arranged)
    wS = nc.dram_tensor("wS", [128, KC * 2 * E], BF16, kind="ExternalInput").ap()
    # partition-major packed outputs: [128, STAGES, JT, 2]; host re-permutes
    idx_out = nc.dram_tensor(
        "idx_out", [128, STAGES * JT * TOP_K], U32, kind="ExternalOutput"
    ).ap()
    w_out = nc.dram_tensor(
        "w_out", [128, STAGES * JT * TOP_K], F32, kind="ExternalOutput"
    ).ap()
    ssum_out = nc.dram_tensor("ssum_out", [1, E], F32, kind="ExternalOutput").ap()

    F32R = mybir.dt.float32r

    with tile.TileContext(nc) as tc, ExitStack() as ctx:
        wpool = ctx.enter_context(tc.tile_pool(name="w", bufs=1))
        xpool = ctx.enter_context(tc.tile_pool(name="x", bufs=2 * KC))
        # PSUM banks: 2 lgT (raw [64,512] logits) + 2 lg2 (transposed) + 1 ssum
        lgtpool = ctx.enter_context(tc.tile_pool(name="lgt", bufs=3, space="PSUM"))
        lg2pool = ctx.enter_context(tc.tile_pool(name="lg2", bufs=2, space="PSUM"))
        sspool = ctx.enter_context(tc.tile_pool(name="ss", bufs=1, space="PSUM"))
        spool = ctx.enter_context(tc.tile_pool(name="st", bufs=2))
        epool = ctx.enter_context(tc.tile_pool(name="e", bufs=2 * JT))

        # stacked gate weight, chunk k at [:, k*2E:(k+1)*2E] (bf16, FWL-able)
        wt_sb = wpool.tile([128, KC * 2 * E], BF16)
        nc.gpsimd.dma_start(wt_sb[:], wS)

        # identity for the PE transpose-back of full [128, 128] blocks
        ident_dram = nc.inline_tensor(np.eye(128, dtype=np.float32), name="ident128")
        ident = wpool.tile([128, 128], F32)
        nc.gpsimd.dma_start(ident[:], ident_dram.ap())

        # output collect tiles: one contiguous DMA each at kernel end
        idx_all = wpool.tile([128, STAGES * JT * 8], U32)
        wq_all = wpool.tile([128, STAGES * JT * TOP_K], F32)

        # whole-kernel softmax column-sum accumulator; allocated full-height
        # so the bank isn't shared with (and zeroed under) the logits tiles
        ssum_t = sspool.tile([128, E], F32)
        ssum = ssum_t[0:1, :]

        # all 32 DMAs issued up front in consumption order: 512KB each,
        # 4KB/partition lines; the full split x (16MB) resides in SBUF
        RD = R // 2
        xks = []
        for ds in range(2):
            for k in range(KC):
                xk = xpool.tile(
                    [128, 2 * RD], BF16, tag="xk", name=f"xk_{ds}_{k}"
                )
                nc.sync.dma_start(
                    xk[:], xsT[k * 128 : (k + 1) * 128, ds, :, :]
                )
                xks.append(xk)

        def emit_mm_half(ds):
            # both stages of this dma-half interleaved per k-chunk, so each
            # arriving 512KB chunk is fully consumed (4 MMs) at once and the
            # last chunk leaves only 4 MMs + one stats chain to drain.
            # 2x2 bf16 split: stationary = stacked [Whi | Wlo] chunk (128
            # cols, one FWL load reused by the hi and lo N=512 streams).
            # PSUM [128, 512]: rows 0:64 = Whi^T x*, rows 64:128 = Wlo^T x*.
            lgTs_pair = [
                lgtpool.tile([128, SR], F32, tag="lgT", name=f"lgT{2 * ds + h}")
                for h in range(2)
            ]
            for k in range(KC):
                stk = wt_sb[:, k * 2 * E : (k + 1) * 2 * E]
                for h in range(2):
                    ro = h * SR
                    xh = xks[ds * KC + k][:, ro : ro + SR]
                    xl = xks[ds * KC + k][:, RD + ro : RD + ro + SR]
                    nc.tensor.matmul(
                        lgTs_pair[h][:],
                        lhsT=stk,
                        rhs=xh,
                        start=(k == 0),
                        stop=False,
                    )
                    nc.tensor.matmul(
                        lgTs_pair[h][:],
                        lhsT=stk,
                        rhs=xl,
                        start=False,
                        stop=(k == KC - 1),
                    )
            return lgTs_pair

        def emit_stats(s, lgT):
            # PSUM -> SBUF; the Whi (rows 0:64) and Wlo (rows 64:128) halves
            # are summed for free by PSUM accumulation across the two
            # transposes-back to [128 rows, 64 experts]
            lgsb = spool.tile([128, SR], F32, tag="lgsb", name=f"lgsb{s}")
            nc.vector.tensor_copy(lgsb[:], lgT[:])
            # transpose whole [128, 128] blocks: row-tile j's transposed block
            # has hi-logits in cols 0:64 and lo-logits in cols 64:128
            lg2 = lg2pool.tile([128, JT * 128], F32, tag="lg2", name=f"lg2_{s}")
            for j in range(JT):
                nc.tensor.matmul(
                    lg2[:, j * 128 : (j + 1) * 128],
                    lhsT=lgsb[:, j * 128 : (j + 1) * 128],
                    rhs=ident[:],
                    is_transpose=True,
                    start=True,
                    stop=True,
                )
            lg2sb = spool.tile([128, JT * 128], F32, tag="lg2sb", name=f"lg2sb{s}")
            nc.vector.tensor_copy(lg2sb[:], lg2[:])
            # hi + lo halves summed in one strided add -> [128, JT*64] logits
            h3 = lg2sb.rearrange("p (j two e) -> p j two e", two=2, e=E)
            lgs = spool.tile([128, JT * E], F32, tag="lgs", name=f"lgs{s}")
            l3 = lgs.rearrange("p (j e) -> p j e", e=E)
            nc.vector.tensor_add(l3[:, :, :], h3[:, :, 0, :], h3[:, :, 1, :])

            vals8 = spool.tile([128, JT * 8], F32, tag="v8", name=f"v8_{s}")
            idx8 = idx_all[:, s * JT * 8 : (s + 1) * JT * 8]
            for j in range(JT):
                nc.vector.max(vals8[:, j * 8 : (j + 1) * 8], lgs[:, j * E : (j + 1) * E])
                nc.vector.max_index(
                    idx8[:, j * 8 : (j + 1) * 8],
                    vals8[:, j * 8 : (j + 1) * 8],
                    lgs[:, j * E : (j + 1) * E],
                )

            v3 = vals8.rearrange("p (j c) -> p j c", c=8)
            l1 = v3[:, :, 0:1]  # [128, JT, 1] top-1 logit
            l2 = v3[:, :, 1:2]  # top-2 logit

            negm1 = spool.tile([128, JT], F32, tag="nm", name=f"nm_{s}")
            nc.vector.tensor_scalar_mul(negm1[:], l1, -1.0)
            d21 = spool.tile([128, JT], F32, tag="d21", name=f"d21_{s}")
            nc.vector.tensor_sub(d21[:], l2, l1)

            denom = spool.tile([128, JT], F32, tag="dn", name=f"dn_{s}")
            ev = []
            for j in range(JT):
                e_j = epool.tile([128, E], F32, tag="ev", name=f"ev_{s}_{j}")
                nc.scalar.activation(
                    e_j[:],
                    lgs[:, j * E : (j + 1) * E],
                    mybir.ActivationFunctionType.Exp,
                    bias=negm1[:, j : j + 1],
                    scale=1.0,
                    accum_out=denom[:, j : j + 1],
                )
                ev.append(e_j)

            recip = spool.tile([128, JT], F32, tag="rc", name=f"rc_{s}")
            nc.vector.reciprocal(recip[:], denom[:])

            # e2 = exp(l2 - l1); w1 = 1/(1+e2); w2 = e2 * w1
            e2 = spool.tile([128, JT], F32, tag="e2", name=f"e2_{s}")
            nc.scalar.activation(e2[:], d21[:], mybir.ActivationFunctionType.Exp)
            den2 = spool.tile([128, JT], F32, tag="dn2", name=f"dn2_{s}")
            nc.vector.tensor_scalar_add(den2[:], e2[:], 1.0)
            wq3 = wq_all[:, s * JT * 2 : (s + 1) * JT * 2].rearrange(
                "p (j c) -> p j c", c=2
            )
            nc.vector.reciprocal(wq3[:, :, 0:1], den2[:])
            nc.vector.tensor_mul(wq3[:, :, 1:2], e2[:], wq3[:, :, 0:1])

            # scores column-sum: ssum += recip_j^T @ e_j  (over all stages)
            for j in range(JT):
                nc.tensor.matmul(
                    ssum[:],
                    lhsT=recip[:, j : j + 1],
                    rhs=ev[j][:],
                    start=(s == 0 and j == 0),
                    stop=(s == STAGES - 1 and j == JT - 1),
                )


        # software pipeline: half 0 (stages 0,1) matmuls, then half 1's
        # matmuls run while half 0's stats drain
        lg01 = emit_mm_half(0)
        emit_stats(0, lg01[0])
        emit_stats(1, lg01[1])
        lg23 = emit_mm_half(1)
        emit_stats(2, lg23[0])
        emit_stats(3, lg23[1])

        # final output DMAs: contiguous 128B partition lines
        i4 = idx_all.rearrange("p (s j c) -> p s j c", s=STAGES, c=8)
        nc.sync.dma_start(
            idx_out.rearrange("p (s j c) -> p s j c", s=STAGES, c=TOP_K),
            i4[:, :, :, 0:TOP_K],
        )
        nc.sync.dma_start(w_out, wq_all[:])
        ssum_sb = spool.tile([1, E], F32)
        nc.vector.tensor_copy(ssum_sb[:], ssum[:])
        nc.sync.dma_start(ssum_out, ssum_sb[:])

    nc.compile()
    return nc


_NC_CACHE = None

# test-harness knobs (harness never touches these; kernel() defaults are fine)
TRACE = False
TMPDIR = None
LAST_RESULT = None


def _get_nc():
    global _NC_CACHE
    if _NC_CACHE is None:
        _NC_CACHE = build_moe_gate_kernel()
    return _NC_CACHE


def kernel(hidden_states: np.ndarray, weight: np.ndarray):
    global LAST_RESULT
    import ml_dtypes

    bf16 = ml_dtypes.bfloat16
    nc = _get_nc()
    x = np.asarray(hidden_states, dtype=np.float32).reshape(B * S, D)
    w = np.asarray(weight, dtype=np.float32)

    # 2x2 bf16 split: v = hi + lo with hi = bf16(v), lo = bf16(v - hi);
    # [Whi | Wlo] stacked so one 128-col stationary serves both terms
    w_hi = w.astype(bf16)
    w_lo = (w - w_hi.astype(np.float32)).astype(bf16)
    wcat = np.concatenate([w_hi.T, w_lo.T], axis=1)  # [D, 2E]
    # SBUF layout: [128, KC*2E] with chunk k at cols [k*2E:(k+1)*2E]
    wS_np = np.ascontiguousarray(
        wcat.reshape(KC, 128, 2 * E).transpose(1, 0, 2).reshape(128, KC * 2 * E)
    )

    x_hi = x.astype(bf16)
    x_lo = (x - x_hi.astype(np.float32)).astype(bf16)

    in_maps = []
    for c in range(N_CORES):
        rows = slice(c * R, (c + 1) * R)
        xs_c = np.ascontiguousarray(
            np.stack(
                [
                    x_hi[rows].T.reshape(D, 2, R // 2),
                    x_lo[rows].T.reshape(D, 2, R // 2),
                ],
                axis=2,
            )
        )  # [D, 2, 2, R/2]
        in_maps.append({"xsT": xs_c, "wS": wS_np})

    res = run_bass_kernel_spmd(
        nc, in_maps, list(range(N_CORES)), trace=TRACE, tmpdir=TMPDIR
    )
    LAST_RESULT = res
    results = res.results

    def unpack(a):
        # [128, STAGES*JT*2] -> rows (s*SR + j*128 + p), cols c
        return (
            a.reshape(128, STAGES, JT, TOP_K)
            .transpose(1, 2, 0, 3)
            .reshape(R, TOP_K)
        )

    idx = np.concatenate(
        [unpack(results[c]["idx_out"]) for c in range(N_CORES)], axis=0
    )
    idx = idx.astype(np.int32)  # values 0..63; uint32 -> int32 exact
    tw = np.concatenate(
        [unpack(results[c]["w_out"]) for c in range(N_CORES)], axis=0
    )
    ssum = np.stack([results[c]["ssum_out"][0] for c in range(N_CORES)])  # [8, E]

    # host-side tiny reductions for the aux loss
    cores_per_batch = N_CORES // B  # 2
    mean_scores = np.zeros((B, E), np.float32)
    for b in range(B):
        mean_scores[b] = (
            ssum[b * cores_per_batch : (b + 1) * cores_per_batch].sum(axis=0) / S
        )
    idx_b = idx.reshape(B, S * TOP_K)
    ce = np.zeros((B, E), np.float32)
    for b in range(B):
        ce[b] = np.bincount(idx_b[b], minlength=E).astype(np.float32)
    ce /= S * TOP_K / E
    aux_loss = np.float32((ce * mean_scores).sum(axis=1).mean() * ALPHA)

    return idx, tw, aux_loss


# revision 24
# speedup vs baseline: 1.9721x; 1.0023x over previous
"""MoE gate (top-2 of 64 experts) Trainium2 Bass kernel.

Problem: hidden_states [4, 4096, 2048] f32, gate weight [64, 2048] f32.
  logits = x @ W.T            [16384, 64]
  scores = softmax(logits)
  topk_w, topk_i = top_k(scores, 2); topk_w normalized by their sum
  aux_loss from per-batch expert counts (ce) and mean scores.

Sharding: data-parallel over batch*seq. 16384 rows -> 2048 rows/core on 8
cores; the [64, 2048] gate weight is replicated (passed pre-transposed).
x is passed per-core pre-transposed ([D, rows] layout) so the contraction
dim D lands on SBUF partitions with fully-contiguous DMA lines.

Per core device program (Tile framework):
  4 stages x 512 rows. Per stage: 16 k-chunk DMAs [128, 512] feed
  fp32 matmuls accumulating logits into one PSUM tile [128, 4*64]
  (4 row-tiles of 128 rows side by side). Stats: DVE max8/max_index give
  top-2 values+indices per row; ACT exp(logits - max) with accum_out gives
  e and its row-sum; softmax column-sums for the aux loss accumulate on the
  PE via a [128,1] x [128,64] matmul with 1/rowsum as the stationary.
  ce and the final scalar aux loss are reduced on host from returned
  per-core indices and score sums (tiny [8,64] + [16384,2] tensors).
"""

import numpy as np
from contextlib import ExitStack

import concourse.bass as bass
import concourse.tile as tile
from concourse import bacc, mybir
from concourse.bass_utils import run_bass_kernel_spmd

# problem constants (hardcoded per harness contract)
B, S, D, E = 4, 4096, 2048, 64
TOP_K = 2
ALPHA = 0.01
N_CORES = 8
R = (B * S) // N_CORES  # 2048 rows per core
KC = D // 128           # 16 contraction chunks
STAGES = 4
SR = R // STAGES        # 512 rows per stage
JT = SR // 128          # 4 row-tiles per stage

F32 = mybir.dt.float32
U32 = mybir.dt.uint32


def build_moe_gate_kernel():
    nc = bacc.Bacc("TRN2", target_bir_lowering=False, debug=False)

    BF16 = mybir.dt.bfloat16
    # hi/lo bf16 split packed flat per d-row: [hiA 1536 | loA 1536 | hiB 512
    # | loB 512]. Group A (stages 0-2) streams with 6KB contiguous partition
    # lines; the small group B (stage 3) arrives last so only ONE stats
    # chain trails the final byte.
    RA = 3 * SR  # 1536 rows in group A
    xsT = nc.dram_tensor("xsT", [D, 2 * R], BF16, kind="ExternalInput").ap()
    # stacked gate weight in SBUF layout [128, KC*2E] (host pre-arranged)
    wS = nc.dram_tensor("wS", [128, KC * 2 * E], BF16, kind="ExternalInput").ap()
    # partition-major packed outputs: [128, STAGES, JT, 2]; host re-permutes
    idx_out = nc.dram_tensor(
        "idx_out", [128, STAGES * JT * TOP_K], U32, kind="ExternalOutput"
    ).ap()
    w_out = nc.dram_tensor(
        "w_out", [128, STAGES * JT * TOP_K], F32, kind="ExternalOutput"
    ).ap()
    ssum_out = nc.dram_tensor("ssum_out", [1, E], F32, kind="ExternalOutput").ap()

    F32R = mybir.dt.float32r

    with tile.TileContext(nc) as tc, ExitStack() as ctx:
        wpool = ctx.enter_context(tc.tile_pool(name="w", bufs=1))
        xpool = ctx.enter_context(tc.tile_pool(name="x", bufs=KC))
        # PSUM banks: 2 lgT (raw [64,512] logits) + 2 lg2 (transposed) + 1 ssum
        lgtpool = ctx.enter_context(tc.tile_pool(name="lgt", bufs=3, space="PSUM"))
        lg2pool = ctx.enter_context(tc.tile_pool(name="lg2", bufs=2, space="PSUM"))
        sspool = ctx.enter_context(tc.tile_pool(name="ss", bufs=1, space="PSUM"))
        spool = ctx.enter_context(tc.tile_pool(name="st", bufs=2))
        epool = ctx.enter_context(tc.tile_pool(name="e", bufs=2 * JT))

        # stacked gate weight, chunk k at [:, k*2E:(k+1)*2E] (bf16, FWL-able)
        wt_sb = wpool.tile([128, KC * 2 * E], BF16)
        nc.gpsimd.dma_start(wt_sb[:], wS)

        # identity for the PE transpose-back of full [128, 128] blocks
        ident_dram = nc.inline_tensor(np.eye(128, dtype=np.float32), name="ident128")
        ident = wpool.tile([128, 128], F32)
        nc.gpsimd.dma_start(ident[:], ident_dram.ap())

        # output collect tiles: one contiguous DMA each at kernel end
        idx_all = wpool.tile([128, STAGES * JT * 8], U32)
        wq_all = wpool.tile([128, STAGES * JT * TOP_K], F32)

        # whole-kernel softmax column-sum accumulator; allocated full-height
        # so the bank isn't shared with (and zeroed under) the logits tiles
        ssum_t = sspool.tile([128, E], F32)
        ssum = ssum_t[0:1, :]

        # all 32 DMAs issued up front in consumption order; the full split x
        # (16MB) resides in SBUF
        xkA, xkB = [], []
        for k in range(KC):
            xa = xpool.tile([128, 2 * RA], BF16, tag="xka", name=f"xka_{k}")
            nc.sync.dma_start(xa[:], xsT[k * 128 : (k + 1) * 128, 0 : 2 * RA])
            xkA.append(xa)
        for k in range(KC):
            xb = xpool.tile([128, 2 * SR], BF16, tag="xkb", name=f"xkb_{k}")
            nc.sync.dma_start(
                xb[:], xsT[k * 128 : (k + 1) * 128, 2 * RA : 2 * R]
            )
            xkB.append(xb)

        def emit_mm_group(tiles, stages, lo_off):
            # all stages of a dma group interleaved per k-chunk, so each
            # arriving chunk is fully consumed at once and the group's last
            # chunk leaves only its stages' final MMs + stats to drain.
            # 2x2 bf16 split: stationary = stacked [Whi | Wlo] chunk (128
            # cols, one FWL load reused by the hi and lo N=512 streams).
            # PSUM [128, 512]: rows 0:64 = Whi^T x*, rows 64:128 = Wlo^T x*.
            lgTs_grp = [
                lgtpool.tile([128, SR], F32, tag="lgT", name=f"lgT{s}")
                for s in stages
            ]
            for k in range(KC):
                stk = wt_sb[:, k * 2 * E : (k + 1) * 2 * E]
                for h in range(len(stages)):
                    ro = h * SR
                    xh = tiles[k][:, ro : ro + SR]
                    xl = tiles[k][:, lo_off + ro : lo_off + ro + SR]
                    nc.tensor.matmul(
                        lgTs_grp[h][:],
                        lhsT=stk,
                        rhs=xh,
                        start=(k == 0),
                        stop=False,
                    )
                    nc.tensor.matmul(
                        lgTs_grp[h][:],
                        lhsT=stk,
                        rhs=xl,
                        start=False,
                        stop=(k == KC - 1),
                    )
            return lgTs_grp

        def emit_stats(s, lgT):
            # PSUM -> SBUF; the Whi (rows 0:64) and Wlo (rows 64:128) halves
            # are summed for free by PSUM accumulation across the two
            # transposes-back to [128 rows, 64 experts]
            lgsb = spool.tile([128, SR], F32, tag="lgsb", name=f"lgsb{s}")
            nc.vector.tensor_copy(lgsb[:], lgT[:])
            # transpose whole [128, 128] blocks: row-tile j's transposed block
            # has hi-logits in cols 0:64 and lo-logits in cols 64:128
            lg2 = lg2pool.tile([128, JT * 128], F32, tag="lg2", name=f"lg2_{s}")
            for j in range(JT):
                nc.tensor.matmul(
                    lg2[:, j * 128 : (j + 1) * 128],
                    lhsT=lgsb[:, j * 128 : (j + 1) * 128],
                    rhs=ident[:],
                    is_transpose=True,
                    start=True,
                    stop=True,
                )
            lg2sb = spool.tile([128, JT * 128], F32, tag="lg2sb", name=f"lg2sb{s}")
            nc.vector.tensor_copy(lg2sb[:], lg2[:])
            # hi + lo halves summed in one strided add -> [128, JT*64] logits
            h3 = lg2sb.rearrange("p (j two e) -> p j two e", two=2, e=E)
            lgs = spool.tile([128, JT * E], F32, tag="lgs", name=f"lgs{s}")
            l3 = lgs.rearrange("p (j e) -> p j e", e=E)
            nc.vector.tensor_add(l3[:, :, :], h3[:, :, 0, :], h3[:, :, 1, :])

            vals8 = spool.tile([128, JT * 8], F32, tag="v8", name=f"v8_{s}")
            idx8 = idx_all[:, s * JT * 8 : (s + 1) * JT * 8]
            for j in range(JT):
                nc.vector.max(vals8[:, j * 8 : (j + 1) * 8], lgs[:, j * E : (j + 1) * E])
                nc.vector.max_index(
                    idx8[:, j * 8 : (j + 1) * 8],
                    vals8[:, j * 8 : (j + 1) * 8],
                    lgs[:, j * E : (j + 1) * E],
                )

            v3 = vals8.rearrange("p (j c) -> p j c", c=8)
            l1 = v3[:, :, 0:1]  # [128, JT, 1] top-1 logit
            l2 = v3[:, :, 1:2]  # top-2 logit

            negm1 = spool.tile([128, JT], F32, tag="nm", name=f"nm_{s}")
            nc.vector.tensor_scalar_mul(negm1[:], l1, -1.0)
            d21 = spool.tile([128, JT], F32, tag="d21", name=f"d21_{s}")
            nc.vector.tensor_sub(d21[:], l2, l1)

            denom = spool.tile([128, JT], F32, tag="dn", name=f"dn_{s}")
            ev = []
            for j in range(JT):
                e_j = epool.tile([128, E], F32, tag="ev", name=f"ev_{s}_{j}")
                nc.scalar.activation(
                    e_j[:],
                    lgs[:, j * E : (j + 1) * E],
                    mybir.ActivationFunctionType.Exp,
                    bias=negm1[:, j : j + 1],
                    scale=1.0,
                    accum_out=denom[:, j : j + 1],
                )
                ev.append(e_j)

            recip = spool.tile([128, JT], F32, tag="rc", name=f"rc_{s}")
            nc.vector.reciprocal(recip[:], denom[:])

            # e2 = exp(l2 - l1); w1 = 1/(1+e2); w2 = e2 * w1
            e2 = spool.tile([128, JT], F32, tag="e2", name=f"e2_{s}")
            nc.scalar.activation(e2[:], d21[:], mybir.ActivationFunctionType.Exp)
            den2 = spool.tile([128, JT], F32, tag="dn2", name=f"dn2_{s}")
            nc.vector.tensor_scalar_add(den2[:], e2[:], 1.0)
            wq3 = wq_all[:, s * JT * 2 : (s + 1) * JT * 2].rearrange(
                "p (j c) -> p j c", c=2
            )
            nc.vector.reciprocal(wq3[:, :, 0:1], den2[:])
            nc.vector.tensor_mul(wq3[:, :, 1:2], e2[:], wq3[:, :, 0:1])

            # scores column-sum: ssum += recip_j^T @ e_j  (over all stages)
            for j in range(JT):
                nc.tensor.matmul(
                    ssum[:],
                    lhsT=recip[:, j : j + 1],
                    rhs=ev[j][:],
                    start=(s == 0 and j == 0),
                    stop=(s == STAGES - 1 and j == JT - 1),
                )


        # software pipeline: group A (stages 0-2) matmuls, then group B's
        # matmuls run while group A's stats drain; only stats(3) trails the
        # last DMA byte
        lgA = emit_mm_group(xkA, [0, 1, 2], RA)
        emit_stats(0, lgA[0])
        emit_stats(1, lgA[1])
        emit_stats(2, lgA[2])
        lgB = emit_mm_group(xkB, [3], SR)
        emit_stats(3, lgB[0])

        # final output DMAs: contiguous 128B partition lines
        i4 = idx_all.rearrange("p (s j c) -> p s j c", s=STAGES, c=8)
        nc.sync.dma_start(
            idx_out.rearrange("p (s j c) -> p s j c", s=STAGES, c=TOP_K),
            i4[:, :, :, 0:TOP_K],
        )
        nc.sync.dma_start(w_out, wq_all[:])
        ssum_sb = spool.tile([1, E], F32)
        nc.vector.tensor_copy(ssum_sb[:], ssum[:])
        nc.sync.dma_start(ssum_out, ssum_sb[:])

    nc.compile()
    return nc


_NC_CACHE = None

# test-harness knobs (harness never touches these; kernel() defaults are fine)
TRACE = False
TMPDIR = None
LAST_RESULT = None


def _get_nc():
    global _NC_CACHE
    if _NC_CACHE is None:
        _NC_CACHE = build_moe_gate_kernel()
    return _NC_CACHE


def kernel(hidden_states: np.ndarray, weight: np.ndarray):
    global LAST_RESULT
    import ml_dtypes

    bf16 = ml_dtypes.bfloat16
    nc = _get_nc()
    x = np.asarray(hidden_states, dtype=np.float32).reshape(B * S, D)
    w = np.asarray(weight, dtype=np.float32)

    # 2x2 bf16 split: v = hi + lo with hi = bf16(v), lo = bf16(v - hi);
    # [Whi | Wlo] stacked so one 128-col stationary serves both terms
    w_hi = w.astype(bf16)
    w_lo = (w - w_hi.astype(np.float32)).astype(bf16)
    wcat = np.concatenate([w_hi.T, w_lo.T], axis=1)  # [D, 2E]
    # SBUF layout: [128, KC*2E] with chunk k at cols [k*2E:(k+1)*2E]
    wS_np = np.ascontiguousarray(
        wcat.reshape(KC, 128, 2 * E).transpose(1, 0, 2).reshape(128, KC * 2 * E)
    )

    x_hi = x.astype(bf16)
    x_lo = (x - x_hi.astype(np.float32)).astype(bf16)

    in_maps = []
    for c in range(N_CORES):
        rows = slice(c * R, (c + 1) * R)
        hiT = x_hi[rows].T
        loT = x_lo[rows].T
        RA = 3 * (R // STAGES)
        xs_c = np.ascontiguousarray(
            np.concatenate(
                [hiT[:, :RA], loT[:, :RA], hiT[:, RA:], loT[:, RA:]], axis=1
            )
        )  # [D, 2R] flat: hiA | loA | hiB | loB
        in_maps.append({"xsT": xs_c, "wS": wS_np})

    res = run_bass_kernel_spmd(
        nc, in_maps, list(range(N_CORES)), trace=TRACE, tmpdir=TMPDIR
    )
    LAST_RESULT = res
    results = res.results

    def unpack(a):
        # [128, STAGES*JT*2] -> rows (s*SR + j*128 + p), cols c
        return (
            a.reshape(128, STAGES, JT, TOP_K)
            .transpose(1, 2, 0, 3)
            .reshape(R, TOP_K)
        )

    idx = np.concatenate(
        [unpack(results[c]["idx_out"]) for c in range(N_CORES)], axis=0
    )
    idx = idx.astype(np.int32)  # values 0..63; uint32 -> int32 exact
    tw = np.concatenate(
        [unpack(results[c]["w_out"]) for c in range(N_CORES)], axis=0
    )
    ssum = np.stack([results[c]["ssum_out"][0] for c in range(N_CORES)])  # [8, E]

    # host-side tiny reductions for the aux loss
    cores_per_batch = N_CORES // B  # 2
    mean_scores = np.zeros((B, E), np.float32)
    for b in range(B):
        mean_scores[b] = (
            ssum[b * cores_per_batch : (b + 1) * cores_per_batch].sum(axis=0) / S
        )
    idx_b = idx.reshape(B, S * TOP_K)
    ce = np.zeros((B, E), np.float32)
    for b in range(B):
        ce[b] = np.bincount(idx_b[b], minlength=E).astype(np.float32)
    ce /= S * TOP_K / E
    aux_loss = np.float32((ce * mean_scores).sum(axis=1).mean() * ALPHA)

    return idx, tw, aux_loss


# revision 25
# speedup vs baseline: 1.9806x; 1.0043x over previous
"""MoE gate (top-2 of 64 experts) Trainium2 Bass kernel.

Problem: hidden_states [4, 4096, 2048] f32, gate weight [64, 2048] f32.
  logits = x @ W.T            [16384, 64]
  scores = softmax(logits)
  topk_w, topk_i = top_k(scores, 2); topk_w normalized by their sum
  aux_loss from per-batch expert counts (ce) and mean scores.

Sharding: data-parallel over batch*seq. 16384 rows -> 2048 rows/core on 8
cores; the [64, 2048] gate weight is replicated (passed pre-transposed).
x is passed per-core pre-transposed ([D, rows] layout) so the contraction
dim D lands on SBUF partitions with fully-contiguous DMA lines.

Per core device program (Tile framework):
  4 stages x 512 rows. Per stage: 16 k-chunk DMAs [128, 512] feed
  fp32 matmuls accumulating logits into one PSUM tile [128, 4*64]
  (4 row-tiles of 128 rows side by side). Stats: DVE max8/max_index give
  top-2 values+indices per row; ACT exp(logits - max) with accum_out gives
  e and its row-sum; softmax column-sums for the aux loss accumulate on the
  PE via a [128,1] x [128,64] matmul with 1/rowsum as the stationary.
  ce and the final scalar aux loss are reduced on host from returned
  per-core indices and score sums (tiny [8,64] + [16384,2] tensors).
"""

import numpy as np
from contextlib import ExitStack

import concourse.bass as bass
import concourse.tile as tile
from concourse import bacc, mybir
from concourse.bass_utils import run_bass_kernel_spmd

# problem constants (hardcoded per harness contract)
B, S, D, E = 4, 4096, 2048, 64
TOP_K = 2
ALPHA = 0.01
N_CORES = 8
R = (B * S) // N_CORES  # 2048 rows per core
KC = D // 128           # 16 contraction chunks
STAGES = 4
SR = R // STAGES        # 512 rows per stage
JT = SR // 128          # 4 row-tiles per stage

F32 = mybir.dt.float32
U32 = mybir.dt.uint32


def build_moe_gate_kernel():
    nc = bacc.Bacc("TRN2", target_bir_lowering=False, debug=False)

    BF16 = mybir.dt.bfloat16
    # hi/lo bf16 split packed flat per d-row: [hiA 1536 | loA 1536 | hiB 512
    # | loB 512]. Group A (stages 0-2) streams with 6KB contiguous partition
    # lines; the small group B (stage 3) arrives last so only ONE stats
    # chain trails the final byte.
    RA = 3 * SR  # 1536 rows in group A
    xsT = nc.dram_tensor("xsT", [D, 2 * R], BF16, kind="ExternalInput").ap()
    # stacked gate weight in SBUF layout [128, KC*2E] (host pre-arranged)
    wS = nc.dram_tensor("wS", [128, KC * 2 * E], BF16, kind="ExternalInput").ap()
    # partition-major packed outputs: [128, STAGES, JT, 2]; host re-permutes
    idx_out = nc.dram_tensor(
        "idx_out", [128, STAGES * JT * TOP_K], U32, kind="ExternalOutput"
    ).ap()
    w_out = nc.dram_tensor(
        "w_out", [128, STAGES * JT * TOP_K], F32, kind="ExternalOutput"
    ).ap()
    ssum_out = nc.dram_tensor("ssum_out", [1, E], F32, kind="ExternalOutput").ap()

    F32R = mybir.dt.float32r

    with tile.TileContext(nc) as tc, ExitStack() as ctx:
        wpool = ctx.enter_context(tc.tile_pool(name="w", bufs=1))
        xpool = ctx.enter_context(tc.tile_pool(name="x", bufs=KC))
        # PSUM banks: 2 lgT (raw [64,512] logits) + 2 lg2 (transposed) + 1 ssum
        lgtpool = ctx.enter_context(tc.tile_pool(name="lgt", bufs=4, space="PSUM"))
        lg2pool = ctx.enter_context(tc.tile_pool(name="lg2", bufs=2, space="PSUM"))
        sspool = ctx.enter_context(tc.tile_pool(name="ss", bufs=1, space="PSUM"))
        spool = ctx.enter_context(tc.tile_pool(name="st", bufs=4))
        epool = ctx.enter_context(tc.tile_pool(name="e", bufs=4 * JT))

        # stacked gate weight, chunk k at [:, k*2E:(k+1)*2E] (bf16, FWL-able)
        wt_sb = wpool.tile([128, KC * 2 * E], BF16)
        nc.gpsimd.dma_start(wt_sb[:], wS)

        # identity for the PE transpose-back of full [128, 128] blocks
        ident_dram = nc.inline_tensor(np.eye(128, dtype=np.float32), name="ident128")
        ident = wpool.tile([128, 128], F32)
        nc.gpsimd.dma_start(ident[:], ident_dram.ap())

        # output collect tiles: one contiguous DMA each at kernel end
        idx_all = wpool.tile([128, STAGES * JT * 8], U32)
        wq_all = wpool.tile([128, STAGES * JT * TOP_K], F32)

        # whole-kernel softmax column-sum accumulator; allocated full-height
        # so the bank isn't shared with (and zeroed under) the logits tiles
        ssum_t = sspool.tile([128, E], F32)
        ssum = ssum_t[0:1, :]

        # all 32 DMAs issued up front in consumption order; the full split x
        # (16MB) resides in SBUF
        xkA, xkB = [], []
        for k in range(KC):
            xa = xpool.tile([128, 2 * RA], BF16, tag="xka", name=f"xka_{k}")
            nc.sync.dma_start(xa[:], xsT[k * 128 : (k + 1) * 128, 0 : 2 * RA])
            xkA.append(xa)
        for k in range(KC):
            xb = xpool.tile([128, 2 * SR], BF16, tag="xkb", name=f"xkb_{k}")
            nc.sync.dma_start(
                xb[:], xsT[k * 128 : (k + 1) * 128, 2 * RA : 2 * R]
            )
            xkB.append(xb)

        def emit_mm_group(tiles, stages, lo_off):
            # all stages of a dma group interleaved per k-chunk, so each
            # arriving chunk is fully consumed at once and the group's last
            # chunk leaves only its stages' final MMs + stats to drain.
            # 2x2 bf16 split: stationary = stacked [Whi | Wlo] chunk (128
            # cols, one FWL load reused by the hi and lo N=512 streams).
            # PSUM [128, 512]: rows 0:64 = Whi^T x*, rows 64:128 = Wlo^T x*.
            lgTs_grp = [
                lgtpool.tile([128, SR], F32, tag="lgT", name=f"lgT{s}")
                for s in stages
            ]
            for k in range(KC):
                stk = wt_sb[:, k * 2 * E : (k + 1) * 2 * E]
                for h in range(len(stages)):
                    ro = h * SR
                    xh = tiles[k][:, ro : ro + SR]
                    xl = tiles[k][:, lo_off + ro : lo_off + ro + SR]
                    nc.tensor.matmul(
                        lgTs_grp[h][:],
                        lhsT=stk,
                        rhs=xh,
                        start=(k == 0),
                        stop=False,
                    )
                    nc.tensor.matmul(
                        lgTs_grp[h][:],
                        lhsT=stk,
                        rhs=xl,
                        start=False,
                        stop=(k == KC - 1),
                    )
            return lgTs_grp

        def emit_stats(s, lgT):
            # PSUM -> SBUF; the Whi (rows 0:64) and Wlo (rows 64:128) halves
            # are summed for free by PSUM accumulation across the two
            # transposes-back to [128 rows, 64 experts]
            lgsb = spool.tile([128, SR], F32, tag="lgsb", name=f"lgsb{s}")
            nc.vector.tensor_copy(lgsb[:], lgT[:])
            # transpose whole [128, 128] blocks: row-tile j's transposed block
            # has hi-logits in cols 0:64 and lo-logits in cols 64:128
            lg2 = lg2pool.tile([128, JT * 128], F32, tag="lg2", name=f"lg2_{s}")
            for j in range(JT):
                nc.tensor.matmul(
                    lg2[:, j * 128 : (j + 1) * 128],
                    lhsT=lgsb[:, j * 128 : (j + 1) * 128],
                    rhs=ident[:],
                    is_transpose=True,
                    start=True,
                    stop=True,
                )
            lg2sb = spool.tile([128, JT * 128], F32, tag="lg2sb", name=f"lg2sb{s}")
            nc.vector.tensor_copy(lg2sb[:], lg2[:])
            # hi + lo halves summed in one strided add -> [128, JT*64] logits
            h3 = lg2sb.rearrange("p (j two e) -> p j two e", two=2, e=E)
            lgs = spool.tile([128, JT * E], F32, tag="lgs", name=f"lgs{s}")
            l3 = lgs.rearrange("p (j e) -> p j e", e=E)
            nc.vector.tensor_add(l3[:, :, :], h3[:, :, 0, :], h3[:, :, 1, :])

            vals8 = spool.tile([128, JT * 8], F32, tag="v8", name=f"v8_{s}")
            idx8 = idx_all[:, s * JT * 8 : (s + 1) * JT * 8]
            for j in range(JT):
                nc.vector.max(vals8[:, j * 8 : (j + 1) * 8], lgs[:, j * E : (j + 1) * E])
                nc.vector.max_index(
                    idx8[:, j * 8 : (j + 1) * 8],
                    vals8[:, j * 8 : (j + 1) * 8],
                    lgs[:, j * E : (j + 1) * E],
                )

            v3 = vals8.rearrange("p (j c) -> p j c", c=8)
            l1 = v3[:, :, 0:1]  # [128, JT, 1] top-1 logit
            l2 = v3[:, :, 1:2]  # top-2 logit

            negm1 = spool.tile([128, JT], F32, tag="nm", name=f"nm_{s}")
            nc.vector.tensor_scalar_mul(negm1[:], l1, -1.0)
            d21 = spool.tile([128, JT], F32, tag="d21", name=f"d21_{s}")
            nc.vector.tensor_sub(d21[:], l2, l1)

            denom = spool.tile([128, JT], F32, tag="dn", name=f"dn_{s}")
            ev = []
            for j in range(JT):
                e_j = epool.tile([128, E], F32, tag="ev", name=f"ev_{s}_{j}")
                nc.scalar.activation(
                    e_j[:],
                    lgs[:, j * E : (j + 1) * E],
                    mybir.ActivationFunctionType.Exp,
                    bias=negm1[:, j : j + 1],
                    scale=1.0,
                    accum_out=denom[:, j : j + 1],
                )
                ev.append(e_j)

            recip = spool.tile([128, JT], F32, tag="rc", name=f"rc_{s}")
            nc.vector.reciprocal(recip[:], denom[:])

            # e2 = exp(l2 - l1); w1 = 1/(1+e2); w2 = e2 * w1
            e2 = spool.tile([128, JT], F32, tag="e2", name=f"e2_{s}")
            nc.scalar.activation(e2[:], d21[:], mybir.ActivationFunctionType.Exp)
            den2 = spool.tile([128, JT], F32, tag="dn2", name=f"dn2_{s}")
            nc.vector.tensor_scalar_add(den2[:], e2[:], 1.0)
            wq3 = wq_all[:, s * JT * 2 : (s + 1) * JT * 2].rearrange(
                "p (j c) -> p j c", c=2
            )
            nc.vector.reciprocal(wq3[:, :, 0:1], den2[:])
            nc.vector.tensor_mul(wq3[:, :, 1:2], e2[:], wq3[:, :, 0:1])

            # scores column-sum: ssum += recip_j^T @ e_j  (over all stages)
            for j in range(JT):
                nc.tensor.matmul(
                    ssum[:],
                    lhsT=recip[:, j : j + 1],
                    rhs=ev[j][:],
                    start=(s == 0 and j == 0),
                    stop=(s == STAGES - 1 and j == JT - 1),
                )


        # software pipeline: group A (stages 0-2) matmuls, then group B's
        # matmuls run while group A's stats drain; only stats(3) trails the
        # last DMA byte
        lgA = emit_mm_group(xkA, [0, 1, 2], RA)
        emit_stats(0, lgA[0])
        emit_stats(1, lgA[1])
        emit_stats(2, lgA[2])
        lgB = emit_mm_group(xkB, [3], SR)
        emit_stats(3, lgB[0])

        # final output DMAs: contiguous 128B partition lines
        i4 = idx_all.rearrange("p (s j c) -> p s j c", s=STAGES, c=8)
        nc.sync.dma_start(
            idx_out.rearrange("p (s j c) -> p s j c", s=STAGES, c=TOP_K),
            i4[:, :, :, 0:TOP_K],
        )
        nc.sync.dma_start(w_out, wq_all[:])
        ssum_sb = spool.tile([1, E], F32)
        nc.vector.tensor_copy(ssum_sb[:], ssum[:])
        nc.sync.dma_start(ssum_out, ssum_sb[:])

    nc.compile()
    return nc


_NC_CACHE = None

# test-harness knobs (harness never touches these; kernel() defaults are fine)
TRACE = False
TMPDIR = None
LAST_RESULT = None


def _get_nc():
    global _NC_CACHE
    if _NC_CACHE is None:
        _NC_CACHE = build_moe_gate_kernel()
    return _NC_CACHE


def kernel(hidden_states: np.ndarray, weight: np.ndarray):
    global LAST_RESULT
    import ml_dtypes

    bf16 = ml_dtypes.bfloat16
    nc = _get_nc()
    x = np.asarray(hidden_states, dtype=np.float32).reshape(B * S, D)
    w = np.asarray(weight, dtype=np.float32)

    # 2x2 bf16 split: v = hi + lo with hi = bf16(v), lo = bf16(v - hi);
    # [Whi | Wlo] stacked so one 128-col stationary serves both terms
    w_hi = w.astype(bf16)
    w_lo = (w - w_hi.astype(np.float32)).astype(bf16)
    wcat = np.concatenate([w_hi.T, w_lo.T], axis=1)  # [D, 2E]
    # SBUF layout: [128, KC*2E] with chunk k at cols [k*2E:(k+1)*2E]
    wS_np = np.ascontiguousarray(
        wcat.reshape(KC, 128, 2 * E).transpose(1, 0, 2).reshape(128, KC * 2 * E)
    )

    x_hi = x.astype(bf16)
    x_lo = (x - x_hi.astype(np.float32)).astype(bf16)

    in_maps = []
    for c in range(N_CORES):
        rows = slice(c * R, (c + 1) * R)
        hiT = x_hi[rows].T
        loT = x_lo[rows].T
        RA = 3 * (R // STAGES)
        xs_c = np.ascontiguousarray(
            np.concatenate(
                [hiT[:, :RA], loT[:, :RA], hiT[:, RA:], loT[:, RA:]], axis=1
            )
        )  # [D, 2R] flat: hiA | loA | hiB | loB
        in_maps.append({"xsT": xs_c, "wS": wS_np})

    res = run_bass_kernel_spmd(
        nc, in_maps, list(range(N_CORES)), trace=TRACE, tmpdir=TMPDIR
    )
    LAST_RESULT = res
    results = res.results

    def unpack(a):
        # [128, STAGES*JT*2] -> rows (s*SR + j*128 + p), cols c
        return (
            a.reshape(128, STAGES, JT, TOP_K)
            .transpose(1, 2, 0, 3)
            .reshape(R, TOP_K)
        )

    idx = np.concatenate(
        [unpack(results[c]["idx_out"]) for c in range(N_CORES)], axis=0
    )
    idx = idx.astype(np.int32)  # values 0..63; uint32 -> int32 exact
    tw = np.concatenate(
        [unpack(results[c]["w_out"]) for c in range(N_CORES)], axis=0
    )
    ssum = np.stack([results[c]["ssum_out"][0] for c in range(N_CORES)])  # [8, E]

    # host-side tiny reductions for the aux loss
    cores_per_batch = N_CORES // B  # 2
    mean_scores = np.zeros((B, E), np.float32)
    for b in range(B):
        mean_scores[b] = (
            ssum[b * cores_per_batch : (b + 1) * cores_per_batch].sum(axis=0) / S
        )
    idx_b = idx.reshape(B, S * TOP_K)
    ce = np.zeros((B, E), np.float32)
    for b in range(B):
        ce[b] = np.bincount(idx_b[b], minlength=E).astype(np.float32)
    ce /= S * TOP_K / E
    aux_loss = np.float32((ce * mean_scores).sum(axis=1).mean() * ALPHA)

    return idx, tw, aux_loss
